# revision 1
# baseline (speedup 1.0000x reference)
"""GATv2 (3-layer) Trainium2 Bass kernel, 8-core node-sharded.

Strategy (per sharding hint): nodes are range-sharded across 8 cores
(1250 each).  Each core computes xl/xr for its own nodes (dense matmuls),
an AllGather replicates xl (the src-side transform) to every core, and each
core processes the edges whose dst falls in its node range (host-side
sorted by dst, tiled into 128-edge tiles grouped by 128-dst blocks).
Per edge tile: dma_gather of xl[src] and xr[dst] (bf16), leaky-relu /
attention-logit / softmax math on DVE+ACT, and a one-hot matmul on the
tensor engine scatter-adds the softmax-weighted features into PSUM per
dst block.  Softmax normalization happens once per dst block (divide by
the gathered exp-sum), followed by residual + layernorm + ELU, and a PE
transpose to feed the next layer's matmuls.
"""

import sys

sys.path.insert(0, "/opt/trn_rl_repo")

import numpy as np

import concourse.bass as bass
import concourse.bacc as bacc
import concourse.tile as tile
import concourse.mybir as mybir
from concourse import bass_utils

# problem constants (hardcoded per spec)
N = 10000
IN_DIM = 512
H, C = 4, 64
HC = 256
HID = 64
NCORES = 8
NPC = N // NCORES          # 1250 nodes per core
NBLK = 10                  # 128-dst blocks per core (1280 padded)
NPAD = NBLK * 128
CH = 8                     # edge tiles per gather chunk (8*128 = 1024 idxs)
EPS = 1e-5

F32 = mybir.dt.float32
BF16 = mybir.dt.bfloat16
NPBF = mybir.dt.np(BF16)

_cache = {}


# ---------------------------------------------------------------- host prep

def _prep_edges(edge_index):
    """Shard edges by dst range, sort by dst, tile into 128-edge tiles
    grouped by 128-dst blocks, pad so every core has identical tile
    structure.  Returns (meta, per_core) where meta = (T_b tuple, flags)."""
    src = np.asarray(edge_index[0], dtype=np.int64)
    dst = np.asarray(edge_index[1], dtype=np.int64)

    per_core_edges = []
    counts = np.zeros((NCORES, NBLK), dtype=np.int64)
    for k in range(NCORES):
        sel = (dst // NPC) == k
        s = src[sel]
        d = dst[sel] - k * NPC
        order = np.argsort(d, kind="stable")
        s, d = s[order], d[order]
        per_core_edges.append((s, d))
        bid = d // 128
        for b in range(NBLK):
            counts[k, b] = int((bid == b).sum())

    T_b = [int(np.ceil(counts[:, b].max() / 128)) for b in range(NBLK)]
    tot = sum(T_b)
    T_b[-1] += (-tot) % CH
    T = sum(T_b)
    base_b = np.cumsum([0] + T_b[:-1])

    per_core = []
    for k in range(NCORES):
        s, d = per_core_edges[k]
        src_ids = np.zeros(T * 128, dtype=np.int16)
        dst_ids = np.zeros(T * 128, dtype=np.int16)
        S = np.zeros((128, T, 128), dtype=NPBF)
        bid = d // 128
        for b in range(NBLK):
            eb = np.nonzero(bid == b)[0]
            sb, db = s[eb], d[eb]
            off = base_b[b] * 128
            j = np.arange(len(eb))
            pos = off + j
            src_ids[pos] = sb.astype(np.int16)
            dst_ids[pos] = db.astype(np.int16)
            S[pos % 128, pos // 128, db - 128 * b] = 1.0
        per_core.append({
            "src_ids": src_ids, "dst_ids": dst_ids, "S": S,
        })

    meta = (tuple(T_b),)
    return meta, per_core


def _wrap_idx(ids):
    """[T*128] -> [128, T*8] int16 in dma_gather layout (idx e at
    partition e%16, col e//16; replicated down the 128 partitions)."""
    arr = ids.reshape(-1, 16).T.astype(np.int16)   # [16, T*8]
    return np.ascontiguousarray(np.tile(arr, (8, 1)))


def _rep(vec):
    return np.ascontiguousarray(np.tile(np.asarray(vec, np.float32).reshape(1, -1), (128, 1)))


# ---------------------------------------------------------------- device kernel

def _build(T_b):
    SKIP = set(globals().get("_SKIP", "").split(","))
    ST = globals().get("_STAGE", 5)
    SUB = globals().get("_SUB", 9)
    REPS = globals().get("_REPS", 1)
    ONECORE = globals().get("_ONECORE", False)
    T = sum(T_b)
    NCHUNK = T // CH
    base_b = np.cumsum([0] + list(T_b[:-1]))
    # per-tile metadata
    blk_of = np.zeros(T, dtype=np.int64)
    first_of = np.zeros(T, dtype=bool)
    last_of = np.zeros(T, dtype=bool)
    for b in range(NBLK):
        blk_of[base_b[b]:base_b[b] + T_b[b]] = b
        first_of[base_b[b]] = True
        last_of[base_b[b] + T_b[b] - 1] = True

    nc = bacc.Bacc("TRN2", target_bir_lowering=False, debug=False,
                   enable_asserts=True, num_devices=1 if ONECORE else NCORES)

    # ---- I/O
    xT_in = nc.dram_tensor("xT_in", [128, 4, NPAD], F32, kind="ExternalInput")
    w_in = {}
    for l, K in ((0, IN_DIM), (1, HC), (2, HC)):
        w_in[f"Wl{l}"] = nc.dram_tensor(f"Wl{l}_in", [K, HC], F32, kind="ExternalInput")
        w_in[f"Wr{l}"] = nc.dram_tensor(f"Wr{l}_in", [K, HC], F32, kind="ExternalInput")
    p0W_in = nc.dram_tensor("p0W_in", [IN_DIM, HC], F32, kind="ExternalInput")
    outW_in = nc.dram_tensor("outW_in", [HC, HID], F32, kind="ExternalInput")
    visW_in = nc.dram_tensor("visW_in", [HID, HID], F32, kind="ExternalInput")
    txtW_in = nc.dram_tensor("txtW_in", [HID, HID], F32, kind="ExternalInput")

    attb_in = nc.dram_tensor("attb_in", [128, 3, HC], BF16, kind="ExternalInput")
    blr_in = nc.dram_tensor("blr_in", [128, 3, HC], F32, kind="ExternalInput")
    brr_in = nc.dram_tensor("brr_in", [128, 3, HC], F32, kind="ExternalInput")
    cbr_in = nc.dram_tensor("cbr_in", [128, 3, HC], F32, kind="ExternalInput")
    gr_in = nc.dram_tensor("gr_in", [128, 3, HC], F32, kind="ExternalInput")
    br2_in = nc.dram_tensor("br2_in", [128, 3, HC], F32, kind="ExternalInput")
    p0br_in = nc.dram_tensor("p0br_in", [128, HC], F32, kind="ExternalInput")
    outbr_in = nc.dram_tensor("outbr_in", [128, HID], F32, kind="ExternalInput")
    visbr_in = nc.dram_tensor("visbr_in", [128, HID], F32, kind="ExternalInput")
    txtbr_in = nc.dram_tensor("txtbr_in", [128, HID], F32, kind="ExternalInput")
    ident_in = nc.dram_tensor("ident_in", [128, 128], F32, kind="ExternalInput")

    isrc_in = nc.dram_tensor("isrc_in", [128, T * 8], mybir.dt.int16, kind="ExternalInput")
    idst_in = nc.dram_tensor("idst_in", [128, T * 8], mybir.dt.int16, kind="ExternalInput")
    S_in = nc.dram_tensor("S_in", [128, T, 128], BF16, kind="ExternalInput")

    o_out = nc.dram_tensor("o_out", [NPC, HID], F32, kind="ExternalOutput")
    v_out = nc.dram_tensor("v_out", [NPC, HID], F32, kind="ExternalOutput")
    t_out = nc.dram_tensor("t_out", [NPC, HID], F32, kind="ExternalOutput")

    # internal DRAM
    xl_stage = nc.dram_tensor("xl_stage", [NPC, HC], BF16, kind="Internal")
    xl_full = nc.dram_tensor("xl_full", [N, HC], BF16, kind="Internal", addr_space="Shared")
    xr_dram = nc.dram_tensor("xr_dram", [NPAD, HC], BF16, kind="Internal")

    with tile.TileContext(nc) as tc:
        with tc.tile_pool(name="const", bufs=1) as cpool, \
             tc.tile_pool(name="state", bufs=1) as spool, \
             tc.tile_pool(name="work", bufs=2) as wpool, \
             tc.tile_pool(name="chunk", bufs=3) as chpool, \
             tc.tile_pool(name="dpsum", bufs=2, space="PSUM") as dpsum, \
             tc.tile_pool(name="apsum", bufs=2, space="PSUM") as apsum, \
             tc.tile_pool(name="tpsum", bufs=2, space="PSUM") as tpsum:

            # ---- load constants
            def ld(pool, dram_ap, shape, dtype):
                t = pool.tile(shape, dtype, tag=dram_ap.tensor.name)
                nc.sync.dma_start(t[:], dram_ap)
                return t

            w_sb = {}
            for l, K in ((0, IN_DIM), (1, HC), (2, HC)):
                KT = K // 128
                for side in ("Wl", "Wr"):
                    ap = w_in[f"{side}{l}"].ap().rearrange("(kc p) f -> p kc f", p=128)
                    w_sb[f"{side}{l}"] = ld(cpool, ap, [128, KT, HC], F32)
            p0W_sb = ld(cpool, p0W_in.ap().rearrange("(kc p) f -> p kc f", p=128), [128, 4, HC], F32)
            outW_sb = ld(cpool, outW_in.ap().rearrange("(kc p) f -> p kc f", p=128), [128, 2, HID], F32)
            visW_sb = ld(cpool, visW_in.ap(), [64, HID], F32)
            txtW_sb = ld(cpool, txtW_in.ap(), [64, HID], F32)
            attb_sb = ld(cpool, attb_in.ap(), [128, 3, HC], BF16)
            blr_sb = ld(cpool, blr_in.ap(), [128, 3, HC], F32)
            brr_sb = ld(cpool, brr_in.ap(), [128, 3, HC], F32)
            cbr_sb = ld(cpool, cbr_in.ap(), [128, 3, HC], F32)
            gr_sb = ld(cpool, gr_in.ap(), [128, 3, HC], F32)
            br2_sb = ld(cpool, br2_in.ap(), [128, 3, HC], F32)
            p0br_sb = ld(cpool, p0br_in.ap(), [128, HC], F32)
            outbr_sb = ld(cpool, outbr_in.ap(), [128, HID], F32)
            visbr_sb = ld(cpool, visbr_in.ap(), [128, HID], F32)
            txtbr_sb = ld(cpool, txtbr_in.ap(), [128, HID], F32)
            ident_sb = ld(cpool, ident_in.ap(), [128, 128], F32)
            isrc_sb = ld(cpool, isrc_in.ap(), [128, T * 8], mybir.dt.int16)
            idst_sb = ld(cpool, idst_in.ap(), [128, T * 8], mybir.dt.int16)

            hTa = spool.tile([128, 2, NPAD], F32, tag="hTa")
            hTb = spool.tile([128, 2, NPAD], F32, tag="hTb")
            resA = spool.tile([128, NBLK, HC], F32, tag="resA")
            resB = spool.tile([128, NBLK, HC], F32, tag="resB")

            hT_cur, res_cur, h_next = None, resA, resB

            for rep in range(REPS):
              hT_cur = None
              for l in range(3 if ST >= 5 else 1):
                K = IN_DIM if l == 0 else HC
                KT = K // 128
                Wl_sb, Wr_sb = w_sb[f"Wl{l}"], w_sb[f"Wr{l}"]

                # ---- dense: xl/xr (+ res for layer 0)
                for rt in range(NBLK):
                    nsl = slice(rt * 128, (rt + 1) * 128)
                    rows = NPC - rt * 128 if rt == NBLK - 1 else 128

                    if l == 0:
                        xtb = wpool.tile([128, 4, 128], F32, tag="xtb",
                                         name=f"xtb_r{rep}_{rt}")
                        nc.sync.dma_start(xtb[:], xT_in.ap()[:, :, nsl])
                        lhs = lambda kc: xtb[:, kc, :]
                    else:
                        lhs = lambda kc: hT_cur[:, kc, nsl]

                    psl = dpsum.tile([128, HC], F32, tag="dps")
                    for kc in range(KT):
                        nc.tensor.matmul(psl[:], lhs(kc), Wl_sb[:, kc, :],
                                         start=(kc == 0), stop=(kc == KT - 1))
                    xl_t = wpool.tile([128, HC], BF16, tag="xlev")
                    nc.vector.tensor_add(xl_t[:], psl[:], blr_sb[:, l, :])
                    nc.sync.dma_start(xl_stage.ap()[rt * 128: rt * 128 + rows, :],
                                      xl_t[0:rows, :])

                    psr = dpsum.tile([128, HC], F32, tag="dps")
                    for kc in range(KT):
                        nc.tensor.matmul(psr[:], lhs(kc), Wr_sb[:, kc, :],
                                         start=(kc == 0), stop=(kc == KT - 1))
                    xr_t = wpool.tile([128, HC], BF16, tag="xrev")
                    nc.vector.tensor_add(xr_t[:], psr[:], brr_sb[:, l, :])
                    nc.sync.dma_start(xr_dram.ap()[nsl, :], xr_t[:])

                    if l == 0:
                        psp = dpsum.tile([128, HC], F32, tag="dps")
                        for kc in range(4):
                            nc.tensor.matmul(psp[:], xtb[:, kc, :], p0W_sb[:, kc, :],
                                             start=(kc == 0), stop=(kc == 3))
                        nc.vector.tensor_add(res_cur[:, rt, :], psp[:], p0br_sb[:])

                # ---- allgather xl
                if ST < 2:
                    break
                if ONECORE or globals().get("_NOAG", 0):
                    nc.gpsimd.dma_start(xl_full.ap()[0:NPC, :], xl_stage.ap())
                else:
                    nc.gpsimd.collective_compute(
                        "AllGather", mybir.AluOpType.bypass,
                        replica_groups=[list(range(NCORES))],
                        ins=[xl_stage.ap()],
                        outs=[xl_full.ap()],
                    )

                # ---- edge phase
                if ST < 3:
                    break
                agg_ps = {}
                for ch in range(NCHUNK):
                    csl = slice(ch * CH * 8, (ch + 1) * CH * 8)
                    g_ch = chpool.tile([128, CH, HC], BF16, tag="gch")
                    nc.gpsimd.dma_gather(g_ch[:], xl_full.ap(), isrc_sb[:, csl],
                                         CH * 128, CH * 128, HC)
                    r_ch = chpool.tile([128, CH, HC], BF16, tag="rch")
                    nc.gpsimd.dma_gather(r_ch[:], xr_dram.ap(), idst_sb[:, csl],
                                         CH * 128, CH * 128, HC)
                    s_ch = chpool.tile([128, CH, 128], BF16, tag="sch")
                    nc.sync.dma_start(s_ch[:], S_in.ap()[:, ch * CH:(ch + 1) * CH, :])

                    if ST < 4:
                        continue
                    y_ch = chpool.tile([128, CH, HC], BF16, tag="ych")
                    if "add" not in SKIP:
                        nc.vector.tensor_add(y_ch[:], g_ch[:], r_ch[:])
                    else:
                        y_ch = g_ch
                    if SUB < 1:
                        continue
                    e_ch = chpool.tile([128, CH, HC], BF16, tag="ech")
                    if "prelu" not in SKIP:
                        nc.scalar.activation(e_ch[:], y_ch[:],
                                             mybir.ActivationFunctionType.Prelu, alpha=0.2)
                    else:
                        e_ch = y_ch
                    if SUB < 2:
                        continue
                    p_ch = chpool.tile([128, CH, HC], BF16, tag="pch")
                    if "pmult" not in SKIP:
                        nc.vector.tensor_tensor(
                            p_ch[:], e_ch[:],
                            attb_sb[:, l:l + 1, :].broadcast_to([128, CH, HC]),
                            mybir.AluOpType.mult)
                    else:
                        p_ch = e_ch
                    # alpha via log-tree of 2x-mode TT adds (plain reduce runs at 1x)
                    al_ch = chpool.tile([128, CH * H], F32, tag="alch")
                    if "notree" in SKIP:
                        nc.vector.tensor_reduce(
                            al_ch[:], p_ch[:].rearrange("p t (h c) -> p (t h) c", h=H),
                            axis=mybir.AxisListType.X, op=mybir.AluOpType.add)
                    elif "alpha" in SKIP:
                        nc.vector.tensor_reduce(
                            al_ch[:], p_ch[:, :, 0:H].rearrange("p t h -> p (t h)", h=H)
                                .rearrange("p (th o) -> p th o", o=1),
                            axis=mybir.AxisListType.X, op=mybir.AluOpType.add)
                    else:
                        p3 = p_ch[:].rearrange("p t (h c) -> p (t h) c", h=H)
                        q1_ch = chpool.tile([128, CH * H, C // 2], BF16, tag="q1ch")
                        nc.vector.tensor_add(q1_ch[:], p3[:, :, 0:32], p3[:, :, 32:64])
                        q2_ch = chpool.tile([128, CH * H, C // 4], BF16, tag="q2ch")
                        nc.vector.tensor_add(q2_ch[:], q1_ch[:, :, 0:16], q1_ch[:, :, 16:32])
                        q3_ch = chpool.tile([128, CH * H, C // 8], BF16, tag="q3ch")
                        nc.vector.tensor_add(q3_ch[:], q2_ch[:, :, 0:8], q2_ch[:, :, 8:16])
                        nc.vector.tensor_reduce(
                            al_ch[:], q3_ch[:],
                            axis=mybir.AxisListType.X, op=mybir.AluOpType.add)
                    if SUB < 3:
                        continue
                    va_ch = chpool.tile([128, CH, HC + H], BF16, tag="vach")
                    if globals().get("_VEXP", 0):
                        # exp broadcast-expanded on ACT (step-0 read), then
                        # 2x-mode in-place multiply by G on DVE
                        nc.scalar.activation(
                            va_ch[:, :, 0:HC].rearrange("p t (h c) -> p t h c", h=H),
                            al_ch[:].rearrange("p (t h o) -> p t h o", h=H, o=1)
                                .broadcast_to([128, CH, H, C]),
                            mybir.ActivationFunctionType.Exp)
                        nc.scalar.activation(
                            va_ch[:, :, HC:HC + H],
                            al_ch[:].rearrange("p (t h) -> p t h", h=H),
                            mybir.ActivationFunctionType.Exp)
                        nc.vector.tensor_tensor(
                            va_ch[:, :, 0:HC], g_ch[:], va_ch[:, :, 0:HC],
                            mybir.AluOpType.mult)
                    else:
                        nc.scalar.activation(
                            va_ch[:, :, HC:HC + H],
                            al_ch[:].rearrange("p (t h) -> p t h", h=H),
                            mybir.ActivationFunctionType.Exp)
                        if "vmult" not in SKIP:
                            nc.vector.tensor_tensor(
                                va_ch[:, :, 0:HC].rearrange("p t (h c) -> p t h c", h=H),
                                g_ch[:].rearrange("p t (h c) -> p t h c", h=H),
                                va_ch[:, :, HC:HC + H]
                                    .rearrange("p t (h c) -> p t h c", c=1)
                                    .broadcast_to([128, CH, H, C]),
                                mybir.AluOpType.mult)

                    if SUB < 4:
                        continue
                    for t in range(CH):
                        gt = ch * CH + t
                        b = int(blk_of[gt])
                        if first_of[gt]:
                            agg_ps[b] = apsum.tile([128, HC + H], F32, tag="aggps",
                                                   name=f"aggps_l{l}_b{b}")
                        if "scatter" not in SKIP:
                            nc.tensor.matmul(agg_ps[b][:], s_ch[:, t, :], va_ch[:, t, :],
                                             start=bool(first_of[gt]), stop=bool(last_of[gt]))
                        elif bool(first_of[gt]):
                            nc.vector.memzero(agg_ps[b][:])

                        if last_of[gt] and SUB >= 5:
                            ps = agg_ps.pop(b)
                            # softmax normalize + conv bias
                            rec_t = wpool.tile([128, H], F32, tag="rec")
                            nc.vector.reciprocal(rec_t[:], ps[:, HC:HC + H])
                            h3_t = wpool.tile([128, HC], F32, tag="h3")
                            nc.vector.tensor_tensor(
                                h3_t[:].rearrange("p (h c) -> p h c", h=H),
                                ps[:, 0:HC].rearrange("p (h c) -> p h c", h=H),
                                rec_t[:].rearrange("p (h c) -> p h c", c=1)
                                    .broadcast_to([128, H, C]),
                                mybir.AluOpType.mult)
                            if SUB < 6:
                                continue
                            nc.vector.tensor_add(h3_t[:], h3_t[:], cbr_sb[:, l, :])
                            nc.vector.tensor_add(h3_t[:], h3_t[:], res_cur[:, b, :])
                            # layernorm
                            if SUB < 7:
                                continue
                            mu_t = wpool.tile([128, 1], F32, tag="mu")
                            nc.vector.tensor_reduce(mu_t[:], h3_t[:],
                                                    axis=mybir.AxisListType.X,
                                                    op=mybir.AluOpType.add)
                            nmu_t = wpool.tile([128, 1], F32, tag="nmu")
                            nc.vector.tensor_scalar_mul(nmu_t[:], mu_t[:], -1.0 / HC)
                            xc_t = wpool.tile([128, HC], F32, tag="xc")
                            nc.scalar.activation(xc_t[:], h3_t[:],
                                                 mybir.ActivationFunctionType.Identity,
                                                 bias=nmu_t[:])
                            sq_t = wpool.tile([128, HC], F32, tag="sq")
                            var_t = wpool.tile([128, 1], F32, tag="var")
                            nc.scalar.activation(sq_t[:], xc_t[:],
                                                 mybir.ActivationFunctionType.Square,
                                                 accum_out=var_t[:])
                            ve_t = wpool.tile([128, 1], F32, tag="ve")
                            nc.vector.tensor_scalar(ve_t[:], var_t[:], 1.0 / HC, EPS,
                                                    mybir.AluOpType.mult,
                                                    mybir.AluOpType.add)
                            # rsqrt via magic-constant + 2 Newton iterations (DVE only;
                            # avoids ACT table switches for Ln/Sqrt)
                            vi_t = wpool.tile([128, 1], mybir.dt.int32, tag="vi")
                            nc.vector.tensor_scalar(
                                vi_t[:], ve_t[:].bitcast(mybir.dt.int32), 1, None,
                                mybir.AluOpType.logical_shift_right)
                            nc.vector.tensor_scalar(vi_t[:], vi_t[:], -1, 0x5f3759df,
                                                    mybir.AluOpType.mult,
                                                    mybir.AluOpType.add)
                            rstd_t = wpool.tile([128, 1], F32, tag="rstd")
                            y_t = vi_t[:].bitcast(F32)
                            t1_t = wpool.tile([128, 1], F32, tag="t1")
                            for _it in range(2):
                                nc.vector.tensor_mul(t1_t[:], y_t, y_t)
                                nc.vector.tensor_mul(t1_t[:], t1_t[:], ve_t[:])
                                nc.vector.tensor_scalar(t1_t[:], t1_t[:], -0.5, 1.5,
                                                        mybir.AluOpType.mult,
                                                        mybir.AluOpType.add)
                                if _it == 0:
                                    nc.vector.tensor_mul(vi_t[:].bitcast(F32), y_t, t1_t[:])
                                else:
                                    nc.vector.tensor_mul(rstd_t[:], y_t, t1_t[:])
                            xn_t = wpool.tile([128, HC], F32, tag="xn")
                            nc.vector.tensor_scalar_mul(xn_t[:], xc_t[:], rstd_t[:])
                            nc.vector.tensor_tensor(xn_t[:], xn_t[:], gr_sb[:, l, :],
                                                    mybir.AluOpType.mult)
                            nc.vector.tensor_add(xn_t[:], xn_t[:], br2_sb[:, l, :])
                            # elu
                            if SUB < 8:
                                continue
                            mn_t = wpool.tile([128, HC], F32, tag="mn")
                            nc.vector.tensor_scalar_min(mn_t[:], xn_t[:], 0.0)
                            ex_t = wpool.tile([128, HC], F32, tag="ex")
                            nc.scalar.activation(ex_t[:], mn_t[:],
                                                 mybir.ActivationFunctionType.Exp)
                            rm_t = wpool.tile([128, HC], F32, tag="rm")
                            nc.vector.tensor_scalar(rm_t[:], xn_t[:], 0.0, -1.0,
                                                    mybir.AluOpType.max,
                                                    mybir.AluOpType.add)
                            nc.vector.tensor_add(h_next[:, b, :], ex_t[:], rm_t[:])
                            # transpose into next layer's hT
                            if SUB < 9:
                                continue
                            if l < 2:
                                hT_next = hTa if hT_cur is not hTa else hTb
                            else:
                                hT_next = hTa if hT_cur is not hTa else hTb
                            for cc in range(2):
                                tp_ps = tpsum.tile([128, 128], F32, tag="tp")
                                nc.tensor.transpose(
                                    tp_ps[:], h_next[:, b, cc * 128:(cc + 1) * 128],
                                    ident_sb[:])
                                nc.vector.tensor_copy(
                                    hT_next[:, cc, b * 128:(b + 1) * 128], tp_ps[:])

                if ST >= 4 and SUB >= 9:
                    hT_cur = hTa if hT_cur is not hTa else hTb
                    res_cur, h_next = h_next, res_cur

            # ---- final heads
            oT_sb = spool.tile([64, NBLK, 128], F32, tag="oT")
            for rt in range(NBLK if hT_cur is not None else 0):
                nsl = slice(rt * 128, (rt + 1) * 128)
                rows = NPC - rt * 128 if rt == NBLK - 1 else 128
                pso = dpsum.tile([128, HID], F32, tag="dps")
                for kc in range(2):
                    nc.tensor.matmul(pso[:], hT_cur[:, kc, nsl], outW_sb[:, kc, :],
                                     start=(kc == 0), stop=(kc == 1))
                o_t = wpool.tile([128, HID], F32, tag="oev")
                nc.vector.tensor_add(o_t[:], pso[:], outbr_sb[:])
                nc.sync.dma_start(o_out.ap()[rt * 128: rt * 128 + rows, :], o_t[0:rows, :])

                tp_ps = tpsum.tile([128, 128], F32, tag="tp")
                nc.tensor.transpose(tp_ps[0:64, :], o_t[:], ident_sb[:])
                nc.vector.tensor_copy(oT_sb[:, rt, :], tp_ps[0:64, :])

                for W_sb, b_sb, dst_dram in ((visW_sb, visbr_sb, v_out),
                                             (txtW_sb, txtbr_sb, t_out)):
                    psv = dpsum.tile([128, HID], F32, tag="dps")
                    nc.tensor.matmul(psv[:], oT_sb[:, rt, :], W_sb[:], start=True, stop=True)
                    vb_t = wpool.tile([128, HID], F32, tag="vbev")
                    nc.vector.tensor_add(vb_t[:], psv[:], b_sb[:])
                    vr_t = wpool.tile([128, HID], F32, tag="vrev")
                    nc.scalar.activation(vr_t[:], vb_t[:],
                                         mybir.ActivationFunctionType.Relu)
                    nc.sync.dma_start(dst_dram.ap()[rt * 128: rt * 128 + rows, :],
                                      vr_t[0:rows, :])

    nc.compile()
    return nc


# ---------------------------------------------------------------- entry point

def prepare(**inputs):
    x = np.asarray(inputs["x"], np.float32)
    edge_index = np.asarray(inputs["edge_index"])

    meta, per_core = _prep_edges(edge_index)
    (T_b,) = meta
    T = sum(T_b)

    key = (meta, globals().get("_STAGE", 5), globals().get("_SUB", 9), globals().get("_VEXP", 0), globals().get("_REPS", 1), globals().get("_NOAG", 0), globals().get("_SKIP", ""))
    if key not in _cache:
        _cache[key] = _build(T_b)
    nc = _cache[key]

    # shared (replicated) inputs
    shared = {}
    for l in range(3):
        shared[f"Wl{l}_in"] = np.asarray(inputs[f"Wl{l}"], np.float32)
        shared[f"Wr{l}_in"] = np.asarray(inputs[f"Wr{l}"], np.float32)
    shared["p0W_in"] = np.asarray(inputs["proj0_W"], np.float32)
    shared["outW_in"] = np.asarray(inputs["out_W"], np.float32)
    shared["visW_in"] = np.asarray(inputs["vis_W"], np.float32)
    shared["txtW_in"] = np.asarray(inputs["txt_W"], np.float32)
    shared["attb_in"] = np.stack(
        [_rep(np.asarray(inputs[f"att{l}"], np.float32).reshape(-1)) for l in range(3)],
        axis=1).astype(NPBF)
    shared["blr_in"] = np.stack([_rep(inputs[f"bl{l}"]) for l in range(3)], axis=1)
    shared["brr_in"] = np.stack([_rep(inputs[f"br{l}"]) for l in range(3)], axis=1)
    shared["cbr_in"] = np.stack([_rep(inputs[f"cb{l}"]) for l in range(3)], axis=1)
    ln_g = np.asarray(inputs["ln_g"], np.float32)
    ln_b = np.asarray(inputs["ln_b"], np.float32)
    shared["gr_in"] = np.stack([_rep(ln_g[l]) for l in range(3)], axis=1)
    shared["br2_in"] = np.stack([_rep(ln_b[l]) for l in range(3)], axis=1)
    shared["p0br_in"] = _rep(inputs["proj0_b"])
    shared["outbr_in"] = _rep(inputs["out_b"])
    shared["visbr_in"] = _rep(inputs["vis_b"])
    shared["txtbr_in"] = _rep(inputs["txt_b"])
    shared["ident_in"] = np.eye(128, dtype=np.float32)

    in_maps = []
    for k in range(NCORES):
        xk = x[k * NPC:(k + 1) * NPC]
        xp = np.zeros((NPAD, IN_DIM), np.float32)
        xp[:NPC] = xk
        xT = np.ascontiguousarray(
            xp.T.reshape(4, 128, NPAD).transpose(1, 0, 2))
        m = dict(shared)
        m["xT_in"] = xT
        m["isrc_in"] = _wrap_idx(per_core[k]["src_ids"])
        m["idst_in"] = _wrap_idx(per_core[k]["dst_ids"])
        m["S_in"] = per_core[k]["S"]
        in_maps.append(m)
    return nc, in_maps


def kernel(**inputs):
    nc, in_maps = prepare(**inputs)
    import os as _os
    _trace = bool(int(_os.environ.get("BASS_KERNEL_TRACE", "0")))
    res = bass_utils.run_bass_kernel_spmd(nc, in_maps, core_ids=list(range(NCORES)),
                                          trace=_trace)

    out = np.concatenate([res.results[k]["o_out"] for k in range(NCORES)], axis=0)
    vis = np.concatenate([res.results[k]["v_out"] for k in range(NCORES)], axis=0)
    txt = np.concatenate([res.results[k]["t_out"] for k in range(NCORES)], axis=0)
    kernel.last_exec_time_ns = res.exec_time_ns
    return out, vis, txt



# revision 5
# speedup vs baseline: 1.4016x; 1.4016x over previous
"""GATv2 (3-layer) Trainium2 Bass kernel, 8-core node-sharded.

Strategy (per sharding hint): nodes are range-sharded across 8 cores
(1250 each, padded to 1280).  Each core computes xl/xr for its own nodes
(dense bf16 matmuls), an AllGather replicates xl (the src-side transform)
to every core, and each core processes the edges whose dst falls in its
node range (host-side sorted by dst, tiled into 128-edge tiles grouped by
128-dst blocks).  Per edge tile: one SWDGE dma_gather fetches xl[src]
(bf16); xr[dst] is NOT gathered — since the tile's dsts all live in one
128-node block, a one-hot transposed-S matmul on the tensor engine
expands xr_block to per-edge rows in PSUM, and an identity-stationary
matmul accumulates the gathered xl on top, so PSUM holds
y = xl[src]+xr[dst] with no DVE add.  Leaky-relu reads PSUM on ACT,
attention logits (att-mult + log-tree reduce) run on DVE, exp expands on
ACT, and a one-hot matmul scatter-adds the softmax-weighted features
into PSUM per dst block.  Softmax normalization happens once per dst
block, followed by residual + layernorm + ELU, a PE transpose feeds the
next layer's matmuls, and (in layer 2) the output heads run per-block so
they overlap the remaining edge chunks.
"""

import sys

sys.path.insert(0, "/opt/trn_rl_repo")

import numpy as np

import concourse.bass as bass
import concourse.bacc as bacc
import concourse.tile as tile
import concourse.mybir as mybir
from concourse import bass_utils

# problem constants (hardcoded per spec)
N = 10000
IN_DIM = 512
H, C = 4, 64
HC = 256
HID = 64
NCORES = 8
NPC = N // NCORES          # 1250 nodes per core
NBLK = 10                  # 128-dst blocks per core (1280 padded)
NPAD = NBLK * 128
CH = 8                     # edge tiles per gather chunk (8*128 = 1024 idxs)
EPS = 1e-5

F32 = mybir.dt.float32
BF16 = mybir.dt.bfloat16
NPBF = mybir.dt.np(BF16)

_cache = {}


# ---------------------------------------------------------------- host prep

def _prep_edges(edge_index):
    """Shard edges by dst range, sort by dst, tile into 128-edge tiles
    grouped by 128-dst blocks, pad so every core has identical tile
    structure.  Returns (meta, per_core)."""
    src = np.asarray(edge_index[0], dtype=np.int64)
    dst = np.asarray(edge_index[1], dtype=np.int64)

    per_core_edges = []
    counts = np.zeros((NCORES, NBLK), dtype=np.int64)
    for k in range(NCORES):
        sel = (dst // NPC) == k
        s = src[sel]
        d = dst[sel] - k * NPC
        order = np.argsort(d, kind="stable")
        s, d = s[order], d[order]
        per_core_edges.append((s, d))
        bid = d // 128
        for b in range(NBLK):
            counts[k, b] = int((bid == b).sum())

    T_b = [int(np.ceil(counts[:, b].max() / 128)) for b in range(NBLK)]
    tot = sum(T_b)
    T_b[-1] += (-tot) % CH
    T = sum(T_b)
    base_b = np.cumsum([0] + T_b[:-1])

    per_core = []
    for k in range(NCORES):
        s, d = per_core_edges[k]
        # remap src to the padded xl_full layout: row = (g//NPC)*NPAD + g%NPC
        s = (s // NPC) * NPAD + (s % NPC)
        src_ids = np.zeros(T * 128, dtype=np.int16)
        S = np.zeros((128, T, 128), dtype=NPBF)
        ST = np.zeros((128, T, 128), dtype=NPBF)
        bid = d // 128
        for b in range(NBLK):
            eb = np.nonzero(bid == b)[0]
            sb, db = s[eb], d[eb]
            off = base_b[b] * 128
            j = np.arange(len(eb))
            pos = off + j
            src_ids[pos] = sb.astype(np.int16)
            S[pos % 128, pos // 128, db - 128 * b] = 1.0
            ST[db - 128 * b, pos // 128, pos % 128] = 1.0
        per_core.append({"src_ids": src_ids, "S": S, "ST": ST})

    meta = (tuple(T_b),)
    return meta, per_core


def _wrap_idx(ids):
    """[T*128] -> [128, T*8] int16 in dma_gather layout (idx e at
    partition e%16, col e//16; replicated down the 128 partitions)."""
    arr = ids.reshape(-1, 16).T.astype(np.int16)   # [16, T*8]
    return np.ascontiguousarray(np.tile(arr, (8, 1)))


def _rep(vec):
    return np.ascontiguousarray(np.tile(np.asarray(vec, np.float32).reshape(1, -1), (128, 1)))


# ---------------------------------------------------------------- device kernel

def _build(T_b):
    T = sum(T_b)
    NCHUNK = T // CH
    base_b = np.cumsum([0] + list(T_b[:-1]))
    # per-tile metadata
    blk_of = np.zeros(T, dtype=np.int64)
    first_of = np.zeros(T, dtype=bool)
    last_of = np.zeros(T, dtype=bool)
    for b in range(NBLK):
        blk_of[base_b[b]:base_b[b] + T_b[b]] = b
        first_of[base_b[b]] = True
        last_of[base_b[b] + T_b[b] - 1] = True

    nc = bacc.Bacc("TRN2", target_bir_lowering=False, debug=False,
                   enable_asserts=True, num_devices=NCORES)

    # ---- I/O
    xT_in = nc.dram_tensor("xT_in", [128, 4, NPAD], BF16, kind="ExternalInput")
    w_in = {}
    for l, K in ((0, IN_DIM), (1, HC), (2, HC)):
        w_in[f"Wl{l}"] = nc.dram_tensor(f"Wl{l}_in", [K, HC], BF16, kind="ExternalInput")
        w_in[f"Wr{l}"] = nc.dram_tensor(f"Wr{l}_in", [K, HC], BF16, kind="ExternalInput")
    p0W_in = nc.dram_tensor("p0W_in", [IN_DIM, HC], BF16, kind="ExternalInput")
    outW_in = nc.dram_tensor("outW_in", [HC, HID], BF16, kind="ExternalInput")
    visW_in = nc.dram_tensor("visW_in", [HID, HID], BF16, kind="ExternalInput")
    txtW_in = nc.dram_tensor("txtW_in", [HID, HID], BF16, kind="ExternalInput")

    attb_in = nc.dram_tensor("attb_in", [128, 3, HC], BF16, kind="ExternalInput")
    blr_in = nc.dram_tensor("blr_in", [128, 3, HC], F32, kind="ExternalInput")
    brr_in = nc.dram_tensor("brr_in", [128, 3, HC], F32, kind="ExternalInput")
    cbr_in = nc.dram_tensor("cbr_in", [128, 3, HC], F32, kind="ExternalInput")
    gr_in = nc.dram_tensor("gr_in", [128, 3, HC], F32, kind="ExternalInput")
    br2_in = nc.dram_tensor("br2_in", [128, 3, HC], F32, kind="ExternalInput")
    p0br_in = nc.dram_tensor("p0br_in", [128, HC], F32, kind="ExternalInput")
    outbr_in = nc.dram_tensor("outbr_in", [128, HID], F32, kind="ExternalInput")
    visbr_in = nc.dram_tensor("visbr_in", [128, HID], F32, kind="ExternalInput")
    txtbr_in = nc.dram_tensor("txtbr_in", [128, HID], F32, kind="ExternalInput")
    identf_in = nc.dram_tensor("identf_in", [128, 128], F32, kind="ExternalInput")
    identb_in = nc.dram_tensor("identb_in", [128, 128], BF16, kind="ExternalInput")

    isrc_in = nc.dram_tensor("isrc_in", [128, T * 8], mybir.dt.int16, kind="ExternalInput")
    S_in = nc.dram_tensor("S_in", [128, T, 128], BF16, kind="ExternalInput")
    ST_in = nc.dram_tensor("ST_in", [128, T, 128], BF16, kind="ExternalInput")

    o_out = nc.dram_tensor("o_out", [NPC, HID], F32, kind="ExternalOutput")
    v_out = nc.dram_tensor("v_out", [NPC, HID], F32, kind="ExternalOutput")
    t_out = nc.dram_tensor("t_out", [NPC, HID], F32, kind="ExternalOutput")

    # internal DRAM
    xl_stage = nc.dram_tensor("xl_stage", [NPAD, HC], BF16, kind="Internal")
    xl_full = nc.dram_tensor("xl_full", [NCORES * NPAD, HC], BF16, kind="Internal",
                             addr_space="Shared")

    with tile.TileContext(nc) as tc:
        with tc.tile_pool(name="const", bufs=1) as cpool, \
             tc.tile_pool(name="state", bufs=1) as spool, \
             tc.tile_pool(name="work", bufs=2) as wpool, \
             tc.tile_pool(name="chunk", bufs=3) as chpool, \
             tc.tile_pool(name="dpsum", bufs=2, space="PSUM") as dpsum, \
             tc.tile_pool(name="apsum", bufs=2, space="PSUM") as apsum, \
             tc.tile_pool(name="ypsum", bufs=2, space="PSUM") as ypsum, \
             tc.tile_pool(name="tpsum", bufs=1, space="PSUM") as tpsum:

            # ---- load constants
            def ld(pool, dram_ap, shape, dtype):
                t = pool.tile(shape, dtype, tag=dram_ap.tensor.name)
                nc.sync.dma_start(t[:], dram_ap)
                return t

            w_sb = {}
            for l, K in ((0, IN_DIM), (1, HC), (2, HC)):
                KT = K // 128
                for side in ("Wl", "Wr"):
                    ap = w_in[f"{side}{l}"].ap().rearrange("(kc p) f -> p kc f", p=128)
                    w_sb[f"{side}{l}"] = ld(cpool, ap, [128, KT, HC], BF16)
            p0W_sb = ld(cpool, p0W_in.ap().rearrange("(kc p) f -> p kc f", p=128), [128, 4, HC], BF16)
            outW_sb = ld(cpool, outW_in.ap().rearrange("(kc p) f -> p kc f", p=128), [128, 2, HID], BF16)
            visW_sb = ld(cpool, visW_in.ap(), [64, HID], BF16)
            txtW_sb = ld(cpool, txtW_in.ap(), [64, HID], BF16)
            attb_sb = ld(cpool, attb_in.ap(), [128, 3, HC], BF16)
            blr_sb = ld(cpool, blr_in.ap(), [128, 3, HC], F32)
            brr_sb = ld(cpool, brr_in.ap(), [128, 3, HC], F32)
            cbr_sb = ld(cpool, cbr_in.ap(), [128, 3, HC], F32)
            gr_sb = ld(cpool, gr_in.ap(), [128, 3, HC], F32)
            br2_sb = ld(cpool, br2_in.ap(), [128, 3, HC], F32)
            p0br_sb = ld(cpool, p0br_in.ap(), [128, HC], F32)
            outbr_sb = ld(cpool, outbr_in.ap(), [128, HID], F32)
            visbr_sb = ld(cpool, visbr_in.ap(), [128, HID], F32)
            txtbr_sb = ld(cpool, txtbr_in.ap(), [128, HID], F32)
            identf_sb = ld(cpool, identf_in.ap(), [128, 128], F32)
            identb_sb = ld(cpool, identb_in.ap(), [128, 128], BF16)
            isrc_sb = ld(cpool, isrc_in.ap(), [128, T * 8], mybir.dt.int16)
            xT_sb = ld(spool, xT_in.ap(), [128, 4, NPAD], BF16)

            hTa = spool.tile([128, 2, NPAD], BF16, tag="hTa")
            hTb = spool.tile([128, 2, NPAD], BF16, tag="hTb")
            resA = spool.tile([128, NBLK, HC], F32, tag="resA")
            resB = spool.tile([128, NBLK, HC], F32, tag="resB")
            xr_sb = spool.tile([128, NBLK, HC], BF16, tag="xr")

            hT_cur, res_cur, h_next = None, resA, resB

            for l in range(3):
                K = IN_DIM if l == 0 else HC
                KT = K // 128
                Wl_sb, Wr_sb = w_sb[f"Wl{l}"], w_sb[f"Wr{l}"]
                lhs_of = (lambda kc, nsl: xT_sb[:, kc, nsl]) if l == 0 else \
                         (lambda kc, nsl, _h=hT_cur: _h[:, kc, nsl])

                # ---- dense xl (first, so the allgather overlaps xr/res)
                for rt in range(NBLK):
                    nsl = slice(rt * 128, (rt + 1) * 128)
                    psl = dpsum.tile([128, HC], F32, tag="dps")
                    for kc in range(KT):
                        nc.tensor.matmul(psl[:], lhs_of(kc, nsl), Wl_sb[:, kc, :],
                                         start=(kc == 0), stop=(kc == KT - 1))
                    xl_t = wpool.tile([128, HC], BF16, tag="xlev")
                    nc.vector.tensor_add(xl_t[:], psl[:], blr_sb[:, l, :])
                    nc.sync.dma_start(xl_stage.ap()[nsl, :], xl_t[:])

                # ---- allgather xl (overlaps the xr/res matmuls below)
                nc.gpsimd.collective_compute(
                    "AllGather", mybir.AluOpType.bypass,
                    replica_groups=[list(range(NCORES))],
                    ins=[xl_stage.ap()],
                    outs=[xl_full.ap()],
                )

                # ---- dense xr (+ res for layer 0)
                for rt in range(NBLK):
                    nsl = slice(rt * 128, (rt + 1) * 128)
                    psr = dpsum.tile([128, HC], F32, tag="dps")
                    for kc in range(KT):
                        nc.tensor.matmul(psr[:], lhs_of(kc, nsl), Wr_sb[:, kc, :],
                                         start=(kc == 0), stop=(kc == KT - 1))
                    nc.vector.tensor_add(xr_sb[:, rt, :], psr[:], brr_sb[:, l, :])

                    if l == 0:
                        psp = dpsum.tile([128, HC], F32, tag="dps")
                        for kc in range(4):
                            nc.tensor.matmul(psp[:], xT_sb[:, kc, nsl], p0W_sb[:, kc, :],
                                             start=(kc == 0), stop=(kc == 3))
                        nc.vector.tensor_add(res_cur[:, rt, :], psp[:], p0br_sb[:])

                # ---- edge phase
                agg_ps = {}
                for ch in range(NCHUNK):
                    csl = slice(ch * CH * 8, (ch + 1) * CH * 8)
                    g_ch = chpool.tile([128, CH, HC], BF16, tag="gch")
                    nc.gpsimd.dma_gather(g_ch[:], xl_full.ap(), isrc_sb[:, csl],
                                         CH * 128, CH * 128, HC)
                    s_ch = chpool.tile([128, CH, 128], BF16, tag="sch")
                    nc.sync.dma_start(s_ch[:], S_in.ap()[:, ch * CH:(ch + 1) * CH, :])
                    st_ch = chpool.tile([128, CH, 128], BF16, tag="stch")
                    nc.sync.dma_start(st_ch[:], ST_in.ap()[:, ch * CH:(ch + 1) * CH, :])

                    # y = xr[dst] (one-hot expand) + xl[src] (identity acc), in PSUM
                    e_ch = chpool.tile([128, CH, HC], BF16, tag="ech")
                    for quarter in range(4):
                        y_ps = ypsum.tile([128, 2, HC], F32, tag="yps")
                        # start=True clears has_written for the WHOLE bank, so
                        # each slice's group must finish before the next starts
                        for j in range(2):
                            t = quarter * 2 + j
                            b = int(blk_of[ch * CH + t])
                            nc.tensor.matmul(y_ps[:, j, :], st_ch[:, t, :],
                                             xr_sb[:, b, :], start=True, stop=False)
                            nc.tensor.matmul(y_ps[:, j, :], identb_sb[:],
                                             g_ch[:, t, :], start=False, stop=True)
                        nc.scalar.activation(
                            e_ch[:, quarter * 2:(quarter + 1) * 2, :], y_ps[:],
                            mybir.ActivationFunctionType.Prelu, alpha=0.2)

                    p_ch = chpool.tile([128, CH, HC], BF16, tag="pch")
                    nc.vector.tensor_tensor(
                        p_ch[:], e_ch[:],
                        attb_sb[:, l:l + 1, :].broadcast_to([128, CH, HC]),
                        mybir.AluOpType.mult)
                    # alpha via log-tree of 2x-mode TT adds (plain reduce runs at 1x)
                    al_ch = chpool.tile([128, CH * H], F32, tag="alch")
                    p3 = p_ch[:].rearrange("p t (h c) -> p (t h) c", h=H)
                    q1_ch = chpool.tile([128, CH * H, C // 2], BF16, tag="q1ch")
                    nc.vector.tensor_add(q1_ch[:], p3[:, :, 0:32], p3[:, :, 32:64])
                    q2_ch = chpool.tile([128, CH * H, C // 4], BF16, tag="q2ch")
                    nc.vector.tensor_add(q2_ch[:], q1_ch[:, :, 0:16], q1_ch[:, :, 16:32])
                    q3_ch = chpool.tile([128, CH * H, C // 8], BF16, tag="q3ch")
                    nc.vector.tensor_add(q3_ch[:], q2_ch[:, :, 0:8], q2_ch[:, :, 8:16])
                    nc.vector.tensor_reduce(
                        al_ch[:], q3_ch[:],
                        axis=mybir.AxisListType.X, op=mybir.AluOpType.add)

                    # exp broadcast-expanded on ACT, then 2x in-place multiply by G
                    va_ch = chpool.tile([128, CH, HC + H], BF16, tag="vach")
                    nc.scalar.activation(
                        va_ch[:, :, 0:HC].rearrange("p t (h c) -> p t h c", h=H),
                        al_ch[:].rearrange("p (t h o) -> p t h o", h=H, o=1)
                            .broadcast_to([128, CH, H, C]),
                        mybir.ActivationFunctionType.Exp)
                    nc.scalar.activation(
                        va_ch[:, :, HC:HC + H],
                        al_ch[:].rearrange("p (t h) -> p t h", h=H),
                        mybir.ActivationFunctionType.Exp)
                    nc.vector.tensor_tensor(
                        va_ch[:, :, 0:HC], g_ch[:], va_ch[:, :, 0:HC],
                        mybir.AluOpType.mult)

                    for t in range(CH):
                        gt = ch * CH + t
                        b = int(blk_of[gt])
                        if first_of[gt]:
                            agg_ps[b] = apsum.tile([128, HC + H], F32, tag="aggps",
                                                   name=f"aggps_l{l}_b{b}")
                        nc.tensor.matmul(agg_ps[b][:], s_ch[:, t, :], va_ch[:, t, :],
                                         start=bool(first_of[gt]), stop=bool(last_of[gt]))

                        if not last_of[gt]:
                            continue
                        ps = agg_ps.pop(b)
                        # softmax normalize + conv bias + residual
                        rec_t = wpool.tile([128, H], F32, tag="rec")
                        nc.vector.reciprocal(rec_t[:], ps[:, HC:HC + H])
                        h3_t = wpool.tile([128, HC], F32, tag="h3")
                        nc.vector.tensor_tensor(
                            h3_t[:].rearrange("p (h c) -> p h c", h=H),
                            ps[:, 0:HC].rearrange("p (h c) -> p h c", h=H),
                            rec_t[:].rearrange("p (h c) -> p h c", c=1)
                                .broadcast_to([128, H, C]),
                            mybir.AluOpType.mult)
                        nc.vector.tensor_add(h3_t[:], h3_t[:], cbr_sb[:, l, :])
                        nc.vector.tensor_add(h3_t[:], h3_t[:], res_cur[:, b, :])
                        # layernorm
                        mu_t = wpool.tile([128, 1], F32, tag="mu")
                        nc.vector.tensor_reduce(mu_t[:], h3_t[:],
                                                axis=mybir.AxisListType.X,
                                                op=mybir.AluOpType.add)
                        nmu_t = wpool.tile([128, 1], F32, tag="nmu")
                        nc.vector.tensor_scalar_mul(nmu_t[:], mu_t[:], -1.0 / HC)
                        xc_t = wpool.tile([128, HC], F32, tag="xc")
                        nc.scalar.activation(xc_t[:], h3_t[:],
                                             mybir.ActivationFunctionType.Identity,
                                             bias=nmu_t[:])
                        sq_t = wpool.tile([128, HC], F32, tag="sq")
                        var_t = wpool.tile([128, 1], F32, tag="var")
                        nc.scalar.activation(sq_t[:], xc_t[:],
                                             mybir.ActivationFunctionType.Square,
                                             accum_out=var_t[:])
                        ve_t = wpool.tile([128, 1], F32, tag="ve")
                        nc.vector.tensor_scalar(ve_t[:], var_t[:], 1.0 / HC, EPS,
                                                mybir.AluOpType.mult,
                                                mybir.AluOpType.add)
                        # rsqrt via magic-constant + 2 Newton iterations (DVE only;
                        # avoids ACT table switches for Ln/Sqrt)
                        vi_t = wpool.tile([128, 1], mybir.dt.int32, tag="vi")
                        nc.vector.tensor_scalar(
                            vi_t[:], ve_t[:].bitcast(mybir.dt.int32), 1, None,
                            mybir.AluOpType.logical_shift_right)
                        nc.vector.tensor_scalar(vi_t[:], vi_t[:], -1, 0x5f3759df,
                                                mybir.AluOpType.mult,
                                                mybir.AluOpType.add)
                        rstd_t = wpool.tile([128, 1], F32, tag="rstd")
                        y_t = vi_t[:].bitcast(F32)
                        t1_t = wpool.tile([128, 1], F32, tag="t1")
                        for _it in range(2):
                            nc.vector.tensor_mul(t1_t[:], y_t, y_t)
                            nc.vector.tensor_mul(t1_t[:], t1_t[:], ve_t[:])
                            nc.vector.tensor_scalar(t1_t[:], t1_t[:], -0.5, 1.5,
                                                    mybir.AluOpType.mult,
                                                    mybir.AluOpType.add)
                            if _it == 0:
                                nc.vector.tensor_mul(vi_t[:].bitcast(F32), y_t, t1_t[:])
                            else:
                                nc.vector.tensor_mul(rstd_t[:], y_t, t1_t[:])
                        xn_t = wpool.tile([128, HC], F32, tag="xn")
                        nc.scalar.activation(xn_t[:], xc_t[:],
                                             mybir.ActivationFunctionType.Identity,
                                             scale=rstd_t[:])
                        nc.vector.tensor_tensor(xn_t[:], xn_t[:], gr_sb[:, l, :],
                                                mybir.AluOpType.mult)
                        nc.vector.tensor_add(xn_t[:], xn_t[:], br2_sb[:, l, :])
                        # elu
                        mn_t = wpool.tile([128, HC], F32, tag="mn")
                        nc.vector.tensor_scalar_min(mn_t[:], xn_t[:], 0.0)
                        ex_t = wpool.tile([128, HC], F32, tag="ex")
                        nc.scalar.activation(ex_t[:], mn_t[:],
                                             mybir.ActivationFunctionType.Exp)
                        rm_t = wpool.tile([128, HC], F32, tag="rm")
                        nc.vector.tensor_scalar(rm_t[:], xn_t[:], 0.0, -1.0,
                                                mybir.AluOpType.max,
                                                mybir.AluOpType.add)
                        nc.vector.tensor_add(h_next[:, b, :], ex_t[:], rm_t[:])
                        # transpose into next layer's hT (bf16 cast on ACT copy)
                        hT_next = hTa if hT_cur is not hTa else hTb
                        for cc in range(2):
                            tp_ps = tpsum.tile([128, 128], F32, tag="tp")
                            nc.tensor.transpose(
                                tp_ps[:], h_next[:, b, cc * 128:(cc + 1) * 128],
                                identf_sb[:])
                            nc.scalar.activation(
                                hT_next[:, cc, b * 128:(b + 1) * 128], tp_ps[:],
                                mybir.ActivationFunctionType.Copy)

                        # ---- final heads per block (layer 2 only; overlaps
                        # the remaining edge chunks)
                        if l == 2:
                            rows = NPC - b * 128 if b == NBLK - 1 else 128
                            nsl2 = slice(b * 128, (b + 1) * 128)
                            pso = dpsum.tile([128, HID], F32, tag="dps")
                            for kc in range(2):
                                nc.tensor.matmul(pso[:], hT_next[:, kc, nsl2],
                                                 outW_sb[:, kc, :],
                                                 start=(kc == 0), stop=(kc == 1))
                            o_t = wpool.tile([128, HID], F32, tag="oev")
                            nc.vector.tensor_add(o_t[:], pso[:], outbr_sb[:])
                            nc.sync.dma_start(
                                o_out.ap()[b * 128: b * 128 + rows, :], o_t[0:rows, :])

                            tp2_ps = tpsum.tile([128, 128], F32, tag="tp")
                            nc.tensor.transpose(tp2_ps[0:64, :], o_t[:], identf_sb[:])
                            oT_t = wpool.tile([64, 128], BF16, tag="oT")
                            nc.scalar.activation(oT_t[:], tp2_ps[0:64, :],
                                                 mybir.ActivationFunctionType.Copy)

                            for W_sb, b_sb, dst_dram in ((visW_sb, visbr_sb, v_out),
                                                         (txtW_sb, txtbr_sb, t_out)):
                                psv = dpsum.tile([128, HID], F32, tag="dps")
                                nc.tensor.matmul(psv[:], oT_t[:], W_sb[:],
                                                 start=True, stop=True)
                                vb_t = wpool.tile([128, HID], F32, tag="vbev")
                                nc.vector.tensor_add(vb_t[:], psv[:], b_sb[:])
                                vr_t = wpool.tile([128, HID], F32, tag="vrev")
                                nc.scalar.activation(vr_t[:], vb_t[:],
                                                     mybir.ActivationFunctionType.Relu)
                                nc.sync.dma_start(
                                    dst_dram.ap()[b * 128: b * 128 + rows, :],
                                    vr_t[0:rows, :])

                hT_cur = hTa if hT_cur is not hTa else hTb
                res_cur, h_next = h_next, res_cur

    nc.compile()
    return nc


# ---------------------------------------------------------------- entry point

def prepare(**inputs):
    x = np.asarray(inputs["x"], np.float32)
    edge_index = np.asarray(inputs["edge_index"])

    meta, per_core = _prep_edges(edge_index)
    (T_b,) = meta
    T = sum(T_b)

    key = meta
    if key not in _cache:
        _cache[key] = _build(T_b)
    nc = _cache[key]

    # shared (replicated) inputs
    shared = {}
    for l in range(3):
        shared[f"Wl{l}_in"] = np.asarray(inputs[f"Wl{l}"], np.float32).astype(NPBF)
        shared[f"Wr{l}_in"] = np.asarray(inputs[f"Wr{l}"], np.float32).astype(NPBF)
    shared["p0W_in"] = np.asarray(inputs["proj0_W"], np.float32).astype(NPBF)
    shared["outW_in"] = np.asarray(inputs["out_W"], np.float32).astype(NPBF)
    shared["visW_in"] = np.asarray(inputs["vis_W"], np.float32).astype(NPBF)
    shared["txtW_in"] = np.asarray(inputs["txt_W"], np.float32).astype(NPBF)
    shared["attb_in"] = np.stack(
        [_rep(np.asarray(inputs[f"att{l}"], np.float32).reshape(-1)) for l in range(3)],
        axis=1).astype(NPBF)
    shared["blr_in"] = np.stack([_rep(inputs[f"bl{l}"]) for l in range(3)], axis=1)
    shared["brr_in"] = np.stack([_rep(inputs[f"br{l}"]) for l in range(3)], axis=1)
    shared["cbr_in"] = np.stack([_rep(inputs[f"cb{l}"]) for l in range(3)], axis=1)
    ln_g = np.asarray(inputs["ln_g"], np.float32)
    ln_b = np.asarray(inputs["ln_b"], np.float32)
    shared["gr_in"] = np.stack([_rep(ln_g[l]) for l in range(3)], axis=1)
    shared["br2_in"] = np.stack([_rep(ln_b[l]) for l in range(3)], axis=1)
    shared["p0br_in"] = _rep(inputs["proj0_b"])
    shared["outbr_in"] = _rep(inputs["out_b"])
    shared["visbr_in"] = _rep(inputs["vis_b"])
    shared["txtbr_in"] = _rep(inputs["txt_b"])
    shared["identf_in"] = np.eye(128, dtype=np.float32)
    shared["identb_in"] = np.eye(128, dtype=np.float32).astype(NPBF)

    in_maps = []
    for k in range(NCORES):
        xk = x[k * NPC:(k + 1) * NPC]
        xp = np.zeros((NPAD, IN_DIM), np.float32)
        xp[:NPC] = xk
        xT = np.ascontiguousarray(
            xp.T.reshape(4, 128, NPAD).transpose(1, 0, 2)).astype(NPBF)
        m = dict(shared)
        m["xT_in"] = xT
        m["isrc_in"] = _wrap_idx(per_core[k]["src_ids"])
        m["S_in"] = per_core[k]["S"]
        m["ST_in"] = per_core[k]["ST"]
        in_maps.append(m)
    return nc, in_maps


def kernel(**inputs):
    nc, in_maps = prepare(**inputs)
    import os as _os
    _trace = bool(int(_os.environ.get("BASS_KERNEL_TRACE", "0")))
    res = bass_utils.run_bass_kernel_spmd(nc, in_maps, core_ids=list(range(NCORES)),
                                          trace=_trace)

    out = np.concatenate([res.results[k]["o_out"] for k in range(NCORES)], axis=0)
    vis = np.concatenate([res.results[k]["v_out"] for k in range(NCORES)], axis=0)
    txt = np.concatenate([res.results[k]["t_out"] for k in range(NCORES)], axis=0)
    kernel.last_exec_time_ns = res.exec_time_ns
    return out, vis, txt


# revision 8
# speedup vs baseline: 1.5154x; 1.0812x over previous
"""GATv2 (3-layer) Trainium2 Bass kernel, 8-core node-sharded.

Strategy (per sharding hint): nodes are range-sharded across 8 cores
(1250 each, padded to 1280).  Each core computes xl/xr for its own nodes
(dense bf16 matmuls), an AllGather replicates xl (the src-side transform)
to every core, and each core processes the edges whose dst falls in its
node range (host-side sorted by dst, tiled into 128-edge tiles grouped by
128-dst blocks).  Per edge tile: one SWDGE dma_gather fetches xl[src]
(bf16); xr[dst] is NOT gathered — since the tile's dsts all live in one
128-node block, a one-hot transposed-S matmul on the tensor engine
expands xr_block to per-edge rows in PSUM, and an identity-stationary
matmul accumulates the gathered xl on top, so PSUM holds
y = xl[src]+xr[dst] with no DVE add.  Leaky-relu reads PSUM on ACT,
attention logits (att-mult + log-tree reduce) run on DVE, exp expands on
ACT, and a one-hot matmul scatter-adds the softmax-weighted features
into PSUM per dst block.  Softmax normalization happens once per dst
block, followed by residual + layernorm + ELU, a PE transpose feeds the
next layer's matmuls, and (in layer 2) the output heads run per-block so
they overlap the remaining edge chunks.
"""

import sys

sys.path.insert(0, "/opt/trn_rl_repo")

import numpy as np

import concourse.bass as bass
import concourse.bacc as bacc
import concourse.tile as tile
import concourse.mybir as mybir
from concourse import bass_utils

# problem constants (hardcoded per spec)
N = 10000
IN_DIM = 512
H, C = 4, 64
HC = 256
HID = 64
NCORES = 8
NPC = N // NCORES          # 1250 nodes per core
NBLK = 10                  # 128-dst blocks per core (1280 padded)
NPAD = NBLK * 128
CH = 8                     # edge tiles per gather chunk (8*128 = 1024 idxs)
EPS = 1e-5

F32 = mybir.dt.float32
BF16 = mybir.dt.bfloat16
NPBF = mybir.dt.np(BF16)

_cache = {}


# ---------------------------------------------------------------- host prep

def _prep_edges(edge_index):
    """Shard edges by dst range, sort by dst, tile into 128-edge tiles
    grouped by 128-dst blocks, pad so every core has identical tile
    structure.  Returns (meta, per_core)."""
    src = np.asarray(edge_index[0], dtype=np.int64)
    dst = np.asarray(edge_index[1], dtype=np.int64)

    per_core_edges = []
    counts = np.zeros((NCORES, NBLK), dtype=np.int64)
    for k in range(NCORES):
        sel = (dst // NPC) == k
        s = src[sel]
        d = dst[sel] - k * NPC
        order = np.argsort(d, kind="stable")
        s, d = s[order], d[order]
        per_core_edges.append((s, d))
        bid = d // 128
        for b in range(NBLK):
            counts[k, b] = int((bid == b).sum())

    T_b = [int(np.ceil(counts[:, b].max() / 128)) for b in range(NBLK)]
    tot = sum(T_b)
    T_b[-1] += (-tot) % CH
    T = sum(T_b)
    base_b = np.cumsum([0] + T_b[:-1])

    per_core = []
    for k in range(NCORES):
        s, d = per_core_edges[k]
        # remap src to the padded xl_full layout: row = (g//NPC)*NPAD + g%NPC
        s = (s // NPC) * NPAD + (s % NPC)
        src_ids = np.zeros(T * 128, dtype=np.int16)
        S = np.zeros((128, T, 128), dtype=NPBF)
        ST = np.zeros((128, T, 128), dtype=NPBF)
        bid = d // 128
        for b in range(NBLK):
            eb = np.nonzero(bid == b)[0]
            sb, db = s[eb], d[eb]
            off = base_b[b] * 128
            j = np.arange(len(eb))
            pos = off + j
            src_ids[pos] = sb.astype(np.int16)
            S[pos % 128, pos // 128, db - 128 * b] = 1.0
            ST[db - 128 * b, pos // 128, pos % 128] = 1.0
        per_core.append({"src_ids": src_ids, "S": S, "ST": ST})

    meta = (tuple(T_b),)
    return meta, per_core


def _wrap_idx(ids):
    """[T*128] -> [128, T*8] int16 in dma_gather layout (idx e at
    partition e%16, col e//16; replicated down the 128 partitions)."""
    arr = ids.reshape(-1, 16).T.astype(np.int16)   # [16, T*8]
    return np.ascontiguousarray(np.tile(arr, (8, 1)))


def _rep(vec):
    return np.ascontiguousarray(np.tile(np.asarray(vec, np.float32).reshape(1, -1), (128, 1)))


# ---------------------------------------------------------------- device kernel

def _build(T_b):
    T = sum(T_b)
    NCHUNK = T // CH
    base_b = np.cumsum([0] + list(T_b[:-1]))
    # per-tile metadata
    blk_of = np.zeros(T, dtype=np.int64)
    first_of = np.zeros(T, dtype=bool)
    last_of = np.zeros(T, dtype=bool)
    for b in range(NBLK):
        blk_of[base_b[b]:base_b[b] + T_b[b]] = b
        first_of[base_b[b]] = True
        last_of[base_b[b] + T_b[b] - 1] = True

    nc = bacc.Bacc("TRN2", target_bir_lowering=False, debug=False,
                   enable_asserts=True, num_devices=NCORES)

    # ---- I/O
    xT_in = nc.dram_tensor("xT_in", [128, 4, NPAD], BF16, kind="ExternalInput")
    w_in = {}
    for l, K in ((0, IN_DIM), (1, HC), (2, HC)):
        w_in[f"Wl{l}"] = nc.dram_tensor(f"Wl{l}_in", [K, HC], BF16, kind="ExternalInput")
        w_in[f"Wr{l}"] = nc.dram_tensor(f"Wr{l}_in", [K, HC], BF16, kind="ExternalInput")
    p0W_in = nc.dram_tensor("p0W_in", [IN_DIM, HC], BF16, kind="ExternalInput")
    outW_in = nc.dram_tensor("outW_in", [HC, HID], BF16, kind="ExternalInput")
    visW_in = nc.dram_tensor("visW_in", [HID, HID], BF16, kind="ExternalInput")
    txtW_in = nc.dram_tensor("txtW_in", [HID, HID], BF16, kind="ExternalInput")

    attb_in = nc.dram_tensor("attb_in", [128, 3, HC], BF16, kind="ExternalInput")
    blr_in = nc.dram_tensor("blr_in", [128, 3, HC], F32, kind="ExternalInput")
    brr_in = nc.dram_tensor("brr_in", [128, 3, HC], F32, kind="ExternalInput")
    cbr_in = nc.dram_tensor("cbr_in", [128, 3, HC], F32, kind="ExternalInput")
    gr_in = nc.dram_tensor("gr_in", [128, 3, HC], F32, kind="ExternalInput")
    br2_in = nc.dram_tensor("br2_in", [128, 3, HC], F32, kind="ExternalInput")
    p0br_in = nc.dram_tensor("p0br_in", [128, HC], F32, kind="ExternalInput")
    outbr_in = nc.dram_tensor("outbr_in", [128, HID], F32, kind="ExternalInput")
    visbr_in = nc.dram_tensor("visbr_in", [128, HID], F32, kind="ExternalInput")
    txtbr_in = nc.dram_tensor("txtbr_in", [128, HID], F32, kind="ExternalInput")
    identf_in = nc.dram_tensor("identf_in", [128, 128], F32, kind="ExternalInput")
    identb_in = nc.dram_tensor("identb_in", [128, 128], BF16, kind="ExternalInput")

    isrc_in = nc.dram_tensor("isrc_in", [128, T * 8], mybir.dt.int16, kind="ExternalInput")
    S_in = nc.dram_tensor("S_in", [128, T, 128], BF16, kind="ExternalInput")
    ST_in = nc.dram_tensor("ST_in", [128, T, 128], BF16, kind="ExternalInput")

    o_out = nc.dram_tensor("o_out", [NPC, HID], F32, kind="ExternalOutput")
    v_out = nc.dram_tensor("v_out", [NPC, HID], F32, kind="ExternalOutput")
    t_out = nc.dram_tensor("t_out", [NPC, HID], F32, kind="ExternalOutput")

    # internal DRAM
    xl_stage = nc.dram_tensor("xl_stage", [NPAD, HC], BF16, kind="Internal")
    xl_full = nc.dram_tensor("xl_full", [NCORES * NPAD, HC], BF16, kind="Internal",
                             addr_space="Shared")

    with tile.TileContext(nc) as tc:
        with tc.tile_pool(name="const", bufs=1) as cpool, \
             tc.tile_pool(name="state", bufs=1) as spool, \
             tc.tile_pool(name="work", bufs=2) as wpool, \
             tc.tile_pool(name="chunk", bufs=4) as chpool, \
             tc.tile_pool(name="dpsum", bufs=2, space="PSUM") as dpsum, \
             tc.tile_pool(name="apsum", bufs=2, space="PSUM") as apsum, \
             tc.tile_pool(name="ypsum", bufs=2, space="PSUM") as ypsum, \
             tc.tile_pool(name="tpsum", bufs=1, space="PSUM") as tpsum:

            # ---- load constants
            def ld(pool, dram_ap, shape, dtype):
                t = pool.tile(shape, dtype, tag=dram_ap.tensor.name)
                nc.sync.dma_start(t[:], dram_ap)
                return t

            w_sb = {}
            for l, K in ((0, IN_DIM), (1, HC), (2, HC)):
                KT = K // 128
                for side in ("Wl", "Wr"):
                    ap = w_in[f"{side}{l}"].ap().rearrange("(kc p) f -> p kc f", p=128)
                    w_sb[f"{side}{l}"] = ld(cpool, ap, [128, KT, HC], BF16)
            p0W_sb = ld(cpool, p0W_in.ap().rearrange("(kc p) f -> p kc f", p=128), [128, 4, HC], BF16)
            outW_sb = ld(cpool, outW_in.ap().rearrange("(kc p) f -> p kc f", p=128), [128, 2, HID], BF16)
            visW_sb = ld(cpool, visW_in.ap(), [64, HID], BF16)
            txtW_sb = ld(cpool, txtW_in.ap(), [64, HID], BF16)
            attb_sb = ld(cpool, attb_in.ap(), [128, 3, HC], BF16)
            blr_sb = ld(cpool, blr_in.ap(), [128, 3, HC], F32)
            brr_sb = ld(cpool, brr_in.ap(), [128, 3, HC], F32)
            cbr_sb = ld(cpool, cbr_in.ap(), [128, 3, HC], F32)
            gr_sb = ld(cpool, gr_in.ap(), [128, 3, HC], F32)
            br2_sb = ld(cpool, br2_in.ap(), [128, 3, HC], F32)
            p0br_sb = ld(cpool, p0br_in.ap(), [128, HC], F32)
            outbr_sb = ld(cpool, outbr_in.ap(), [128, HID], F32)
            visbr_sb = ld(cpool, visbr_in.ap(), [128, HID], F32)
            txtbr_sb = ld(cpool, txtbr_in.ap(), [128, HID], F32)
            identf_sb = ld(cpool, identf_in.ap(), [128, 128], F32)
            identb_sb = ld(cpool, identb_in.ap(), [128, 128], BF16)
            isrc_sb = ld(cpool, isrc_in.ap(), [128, T * 8], mybir.dt.int16)
            xT_sb = ld(spool, xT_in.ap(), [128, 4, NPAD], BF16)

            hTa = spool.tile([128, 2, NPAD], BF16, tag="hTa")
            hTb = spool.tile([128, 2, NPAD], BF16, tag="hTb")
            resA = spool.tile([128, NBLK, HC], F32, tag="resA")
            resB = spool.tile([128, NBLK, HC], F32, tag="resB")
            xr_sb = spool.tile([128, NBLK, HC], BF16, tag="xr")

            hT_cur, res_cur, h_next = None, resA, resB

            for l in range(3):
                K = IN_DIM if l == 0 else HC
                KT = K // 128
                Wl_sb, Wr_sb = w_sb[f"Wl{l}"], w_sb[f"Wr{l}"]
                lhs_of = (lambda kc, nsl: xT_sb[:, kc, nsl]) if l == 0 else \
                         (lambda kc, nsl, _h=hT_cur: _h[:, kc, nsl])

                # ---- dense xl (first, so the allgather overlaps xr/res)
                for rt in range(NBLK):
                    nsl = slice(rt * 128, (rt + 1) * 128)
                    psl = dpsum.tile([128, HC], F32, tag="dps")
                    for kc in range(KT):
                        nc.tensor.matmul(psl[:], lhs_of(kc, nsl), Wl_sb[:, kc, :],
                                         start=(kc == 0), stop=(kc == KT - 1))
                    xl_t = wpool.tile([128, HC], BF16, tag="xlev")
                    nc.vector.tensor_add(xl_t[:], psl[:], blr_sb[:, l, :])
                    nc.sync.dma_start(xl_stage.ap()[nsl, :], xl_t[:])

                # ---- allgather xl (overlaps the xr/res matmuls below)
                nc.gpsimd.collective_compute(
                    "AllGather", mybir.AluOpType.bypass,
                    replica_groups=[list(range(NCORES))],
                    ins=[xl_stage.ap()],
                    outs=[xl_full.ap()],
                )

                # ---- dense xr (+ res for layer 0)
                for rt in range(NBLK):
                    nsl = slice(rt * 128, (rt + 1) * 128)
                    psr = dpsum.tile([128, HC], F32, tag="dps")
                    for kc in range(KT):
                        nc.tensor.matmul(psr[:], lhs_of(kc, nsl), Wr_sb[:, kc, :],
                                         start=(kc == 0), stop=(kc == KT - 1))
                    nc.vector.tensor_add(xr_sb[:, rt, :], psr[:], brr_sb[:, l, :])

                    if l == 0:
                        psp = dpsum.tile([128, HC], F32, tag="dps")
                        for kc in range(4):
                            nc.tensor.matmul(psp[:], xT_sb[:, kc, nsl], p0W_sb[:, kc, :],
                                             start=(kc == 0), stop=(kc == 3))
                        nc.vector.tensor_add(res_cur[:, rt, :], psp[:], p0br_sb[:])

                # ---- edge phase
                agg_ps = {}
                for ch in range(NCHUNK):
                    csl = slice(ch * CH * 8, (ch + 1) * CH * 8)
                    g_ch = chpool.tile([128, CH, HC], BF16, tag="gch")
                    nc.gpsimd.dma_gather(g_ch[:], xl_full.ap(), isrc_sb[:, csl],
                                         CH * 128, CH * 128, HC)
                    s_ch = chpool.tile([128, CH, 128], BF16, tag="sch")
                    nc.sync.dma_start(s_ch[:], S_in.ap()[:, ch * CH:(ch + 1) * CH, :])
                    st_ch = chpool.tile([128, CH, 128], BF16, tag="stch")
                    nc.sync.dma_start(st_ch[:], ST_in.ap()[:, ch * CH:(ch + 1) * CH, :])

                    # y = xr[dst] (one-hot expand) + xl[src] (identity acc), in PSUM
                    e_ch = chpool.tile([128, CH, HC], BF16, tag="ech")
                    for quarter in range(4):
                        y_ps = ypsum.tile([128, 2, HC], F32, tag="yps")
                        # start=True clears has_written for the WHOLE bank, so
                        # each slice's group must finish before the next starts
                        for j in range(2):
                            t = quarter * 2 + j
                            b = int(blk_of[ch * CH + t])
                            nc.tensor.matmul(y_ps[:, j, :], st_ch[:, t, :],
                                             xr_sb[:, b, :], start=True, stop=False)
                            nc.tensor.matmul(y_ps[:, j, :], identb_sb[:],
                                             g_ch[:, t, :], start=False, stop=True)
                        nc.scalar.activation(
                            e_ch[:, quarter * 2:(quarter + 1) * 2, :], y_ps[:],
                            mybir.ActivationFunctionType.Prelu, alpha=0.2)

                    nc.vector.tensor_tensor(
                        e_ch[:], e_ch[:],
                        attb_sb[:, l:l + 1, :].broadcast_to([128, CH, HC]),
                        mybir.AluOpType.mult)
                    # alpha via log-tree of 2x-mode TT adds (plain reduce runs at 1x)
                    al_ch = chpool.tile([128, CH * H], F32, tag="alch")
                    p3 = e_ch[:].rearrange("p t (h c) -> p (t h) c", h=H)
                    q1_ch = chpool.tile([128, CH * H, C // 2], BF16, tag="q1ch")
                    nc.vector.tensor_add(q1_ch[:], p3[:, :, 0:32], p3[:, :, 32:64])
                    q2_ch = chpool.tile([128, CH * H, C // 4], BF16, tag="q2ch")
                    nc.vector.tensor_add(q2_ch[:], q1_ch[:, :, 0:16], q1_ch[:, :, 16:32])
                    q3_ch = chpool.tile([128, CH * H, C // 8], BF16, tag="q3ch")
                    nc.vector.tensor_add(q3_ch[:], q2_ch[:, :, 0:8], q2_ch[:, :, 8:16])
                    nc.vector.tensor_reduce(
                        al_ch[:], q3_ch[:],
                        axis=mybir.AxisListType.X, op=mybir.AluOpType.add)

                    # exp broadcast-expanded on ACT, then 2x in-place multiply by G
                    va_ch = chpool.tile([128, CH, HC + H], BF16, tag="vach")
                    nc.scalar.activation(
                        va_ch[:, :, 0:HC].rearrange("p t (h c) -> p t h c", h=H),
                        al_ch[:].rearrange("p (t h o) -> p t h o", h=H, o=1)
                            .broadcast_to([128, CH, H, C]),
                        mybir.ActivationFunctionType.Exp)
                    nc.scalar.activation(
                        va_ch[:, :, HC:HC + H],
                        al_ch[:].rearrange("p (t h) -> p t h", h=H),
                        mybir.ActivationFunctionType.Exp)
                    nc.vector.tensor_tensor(
                        va_ch[:, :, 0:HC], g_ch[:], va_ch[:, :, 0:HC],
                        mybir.AluOpType.mult)

                    for t in range(CH):
                        gt = ch * CH + t
                        b = int(blk_of[gt])
                        if first_of[gt]:
                            agg_ps[b] = apsum.tile([128, HC + H], F32, tag="aggps",
                                                   name=f"aggps_l{l}_b{b}")
                        nc.tensor.matmul(agg_ps[b][:], s_ch[:, t, :], va_ch[:, t, :],
                                         start=bool(first_of[gt]), stop=bool(last_of[gt]))

                        if not last_of[gt]:
                            continue
                        ps = agg_ps.pop(b)
                        # softmax normalize + conv bias + residual
                        rec_t = wpool.tile([128, H], F32, tag="rec")
                        nc.vector.reciprocal(rec_t[:], ps[:, HC:HC + H])
                        h3_t = wpool.tile([128, HC], F32, tag="h3")
                        nc.vector.tensor_tensor(
                            h3_t[:].rearrange("p (h c) -> p h c", h=H),
                            ps[:, 0:HC].rearrange("p (h c) -> p h c", h=H),
                            rec_t[:].rearrange("p (h c) -> p h c", c=1)
                                .broadcast_to([128, H, C]),
                            mybir.AluOpType.mult)
                        nc.vector.tensor_add(h3_t[:], h3_t[:], cbr_sb[:, l, :])
                        nc.vector.tensor_add(h3_t[:], h3_t[:], res_cur[:, b, :])
                        # layernorm
                        mu_t = wpool.tile([128, 1], F32, tag="mu")
                        nc.vector.tensor_reduce(mu_t[:], h3_t[:],
                                                axis=mybir.AxisListType.X,
                                                op=mybir.AluOpType.add)
                        nmu_t = wpool.tile([128, 1], F32, tag="nmu")
                        nc.vector.tensor_scalar_mul(nmu_t[:], mu_t[:], -1.0 / HC)
                        xc_t = wpool.tile([128, HC], F32, tag="xc")
                        nc.scalar.activation(xc_t[:], h3_t[:],
                                             mybir.ActivationFunctionType.Identity,
                                             bias=nmu_t[:])
                        sq_t = wpool.tile([128, HC], F32, tag="sq")
                        var_t = wpool.tile([128, 1], F32, tag="var")
                        nc.scalar.activation(sq_t[:], xc_t[:],
                                             mybir.ActivationFunctionType.Square,
                                             accum_out=var_t[:])
                        ve_t = wpool.tile([128, 1], F32, tag="ve")
                        nc.vector.tensor_scalar(ve_t[:], var_t[:], 1.0 / HC, EPS,
                                                mybir.AluOpType.mult,
                                                mybir.AluOpType.add)
                        # rsqrt via magic-constant + 2 Newton iterations (DVE only;
                        # avoids ACT table switches for Ln/Sqrt)
                        vi_t = wpool.tile([128, 1], mybir.dt.int32, tag="vi")
                        nc.vector.tensor_scalar(
                            vi_t[:], ve_t[:].bitcast(mybir.dt.int32), 1, None,
                            mybir.AluOpType.logical_shift_right)
                        nc.vector.tensor_scalar(vi_t[:], vi_t[:], -1, 0x5f3759df,
                                                mybir.AluOpType.mult,
                                                mybir.AluOpType.add)
                        rstd_t = wpool.tile([128, 1], F32, tag="rstd")
                        y_t = vi_t[:].bitcast(F32)
                        t1_t = wpool.tile([128, 1], F32, tag="t1")
                        for _it in range(2):
                            nc.vector.tensor_mul(t1_t[:], y_t, y_t)
                            nc.vector.tensor_mul(t1_t[:], t1_t[:], ve_t[:])
                            nc.vector.tensor_scalar(t1_t[:], t1_t[:], -0.5, 1.5,
                                                    mybir.AluOpType.mult,
                                                    mybir.AluOpType.add)
                            if _it == 0:
                                nc.vector.tensor_mul(vi_t[:].bitcast(F32), y_t, t1_t[:])
                            else:
                                nc.vector.tensor_mul(rstd_t[:], y_t, t1_t[:])
                        xn_t = wpool.tile([128, HC], F32, tag="xn")
                        nc.scalar.activation(xn_t[:], xc_t[:],
                                             mybir.ActivationFunctionType.Identity,
                                             scale=rstd_t[:])
                        nc.vector.tensor_tensor(xn_t[:], xn_t[:], gr_sb[:, l, :],
                                                mybir.AluOpType.mult)
                        nc.vector.tensor_add(xn_t[:], xn_t[:], br2_sb[:, l, :])
                        # elu
                        mn_t = wpool.tile([128, HC], F32, tag="mn")
                        nc.vector.tensor_scalar_min(mn_t[:], xn_t[:], 0.0)
                        ex_t = wpool.tile([128, HC], F32, tag="ex")
                        nc.scalar.activation(ex_t[:], mn_t[:],
                                             mybir.ActivationFunctionType.Exp)
                        rm_t = wpool.tile([128, HC], F32, tag="rm")
                        nc.vector.tensor_scalar(rm_t[:], xn_t[:], 0.0, -1.0,
                                                mybir.AluOpType.max,
                                                mybir.AluOpType.add)
                        nc.vector.tensor_add(h_next[:, b, :], ex_t[:], rm_t[:])
                        # transpose into next layer's hT (bf16 cast on ACT copy)
                        hT_next = hTa if hT_cur is not hTa else hTb
                        for cc in range(2):
                            tp_ps = tpsum.tile([128, 128], F32, tag="tp")
                            nc.tensor.transpose(
                                tp_ps[:], h_next[:, b, cc * 128:(cc + 1) * 128],
                                identf_sb[:])
                            nc.scalar.activation(
                                hT_next[:, cc, b * 128:(b + 1) * 128], tp_ps[:],
                                mybir.ActivationFunctionType.Copy)

                        # ---- final heads per block (layer 2 only; overlaps
                        # the remaining edge chunks)
                        if l == 2:
                            rows = NPC - b * 128 if b == NBLK - 1 else 128
                            nsl2 = slice(b * 128, (b + 1) * 128)
                            pso = dpsum.tile([128, HID], F32, tag="dps")
                            for kc in range(2):
                                nc.tensor.matmul(pso[:], hT_next[:, kc, nsl2],
                                                 outW_sb[:, kc, :],
                                                 start=(kc == 0), stop=(kc == 1))
                            o_t = wpool.tile([128, HID], F32, tag="oev")
                            nc.vector.tensor_add(o_t[:], pso[:], outbr_sb[:])
                            nc.sync.dma_start(
                                o_out.ap()[b * 128: b * 128 + rows, :], o_t[0:rows, :])

                            tp2_ps = tpsum.tile([128, 128], F32, tag="tp")
                            nc.tensor.transpose(tp2_ps[0:64, :], o_t[:], identf_sb[:])
                            oT_t = wpool.tile([64, 128], BF16, tag="oT")
                            nc.scalar.activation(oT_t[:], tp2_ps[0:64, :],
                                                 mybir.ActivationFunctionType.Copy)

                            for W_sb, b_sb, dst_dram in ((visW_sb, visbr_sb, v_out),
                                                         (txtW_sb, txtbr_sb, t_out)):
                                psv = dpsum.tile([128, HID], F32, tag="dps")
                                nc.tensor.matmul(psv[:], oT_t[:], W_sb[:],
                                                 start=True, stop=True)
                                vb_t = wpool.tile([128, HID], F32, tag="vbev")
                                nc.vector.tensor_add(vb_t[:], psv[:], b_sb[:])
                                vr_t = wpool.tile([128, HID], F32, tag="vrev")
                                nc.scalar.activation(vr_t[:], vb_t[:],
                                                     mybir.ActivationFunctionType.Relu)
                                nc.sync.dma_start(
                                    dst_dram.ap()[b * 128: b * 128 + rows, :],
                                    vr_t[0:rows, :])

                hT_cur = hTa if hT_cur is not hTa else hTb
                res_cur, h_next = h_next, res_cur

    nc.compile()
    return nc


# ---------------------------------------------------------------- entry point

def prepare(**inputs):
    x = np.asarray(inputs["x"], np.float32)
    edge_index = np.asarray(inputs["edge_index"])

    meta, per_core = _prep_edges(edge_index)
    (T_b,) = meta
    T = sum(T_b)

    key = meta
    if key not in _cache:
        _cache[key] = _build(T_b)
    nc = _cache[key]

    # shared (replicated) inputs
    shared = {}
    for l in range(3):
        shared[f"Wl{l}_in"] = np.asarray(inputs[f"Wl{l}"], np.float32).astype(NPBF)
        shared[f"Wr{l}_in"] = np.asarray(inputs[f"Wr{l}"], np.float32).astype(NPBF)
    shared["p0W_in"] = np.asarray(inputs["proj0_W"], np.float32).astype(NPBF)
    shared["outW_in"] = np.asarray(inputs["out_W"], np.float32).astype(NPBF)
    shared["visW_in"] = np.asarray(inputs["vis_W"], np.float32).astype(NPBF)
    shared["txtW_in"] = np.asarray(inputs["txt_W"], np.float32).astype(NPBF)
    shared["attb_in"] = np.stack(
        [_rep(np.asarray(inputs[f"att{l}"], np.float32).reshape(-1)) for l in range(3)],
        axis=1).astype(NPBF)
    shared["blr_in"] = np.stack([_rep(inputs[f"bl{l}"]) for l in range(3)], axis=1)
    shared["brr_in"] = np.stack([_rep(inputs[f"br{l}"]) for l in range(3)], axis=1)
    shared["cbr_in"] = np.stack([_rep(inputs[f"cb{l}"]) for l in range(3)], axis=1)
    ln_g = np.asarray(inputs["ln_g"], np.float32)
    ln_b = np.asarray(inputs["ln_b"], np.float32)
    shared["gr_in"] = np.stack([_rep(ln_g[l]) for l in range(3)], axis=1)
    shared["br2_in"] = np.stack([_rep(ln_b[l]) for l in range(3)], axis=1)
    shared["p0br_in"] = _rep(inputs["proj0_b"])
    shared["outbr_in"] = _rep(inputs["out_b"])
    shared["visbr_in"] = _rep(inputs["vis_b"])
    shared["txtbr_in"] = _rep(inputs["txt_b"])
    shared["identf_in"] = np.eye(128, dtype=np.float32)
    shared["identb_in"] = np.eye(128, dtype=np.float32).astype(NPBF)

    in_maps = []
    for k in range(NCORES):
        xk = x[k * NPC:(k + 1) * NPC]
        xp = np.zeros((NPAD, IN_DIM), np.float32)
        xp[:NPC] = xk
        xT = np.ascontiguousarray(
            xp.T.reshape(4, 128, NPAD).transpose(1, 0, 2)).astype(NPBF)
        m = dict(shared)
        m["xT_in"] = xT
        m["isrc_in"] = _wrap_idx(per_core[k]["src_ids"])
        m["S_in"] = per_core[k]["S"]
        m["ST_in"] = per_core[k]["ST"]
        in_maps.append(m)
    return nc, in_maps


def kernel(**inputs):
    nc, in_maps = prepare(**inputs)
    import os as _os
    _trace = bool(int(_os.environ.get("BASS_KERNEL_TRACE", "0")))
    res = bass_utils.run_bass_kernel_spmd(nc, in_maps, core_ids=list(range(NCORES)),
                                          trace=_trace)

    out = np.concatenate([res.results[k]["o_out"] for k in range(NCORES)], axis=0)
    vis = np.concatenate([res.results[k]["v_out"] for k in range(NCORES)], axis=0)
    txt = np.concatenate([res.results[k]["t_out"] for k in range(NCORES)], axis=0)
    kernel.last_exec_time_ns = res.exec_time_ns
    return out, vis, txt


# revision 12
# speedup vs baseline: 1.7435x; 1.1506x over previous
"""GATv2 (3-layer) Trainium2 Bass kernel, 8-core node-sharded.

Strategy (per sharding hint): nodes are range-sharded across 8 cores
(1250 each, padded to 1280).  Each core computes xl/xr for its own nodes
(dense bf16 matmuls), per-128-node-block AllGathers replicate xl (the
src-side transform) to every core, and each core processes the edges
whose dst falls in its node range (host-side sorted by dst, tiled into
128-edge tiles grouped by 128-dst blocks).  Per edge tile: one SWDGE
dma_gather fetches xl[src] (bf16); xr[dst] is NOT gathered — since the
tile's dsts all live in one 128-node block, a one-hot transposed-S
matmul (fp8 stationary) on the tensor engine expands xr_block to
per-edge rows in PSUM, and an identity-stationary matmul accumulates the
gathered xl on top, so PSUM holds y = xl[src]+xr[dst] with no DVE add.
Leaky-relu reads PSUM on ACT, attention logits (att-mult + log-tree
reduce) run on DVE, exp expands on ACT, and a one-hot matmul scatter-
adds the softmax-weighted features into PSUM per dst block.  Softmax
normalization happens once per dst block, followed by residual +
layernorm + ELU, a PE transpose, and then — inside the same per-block
completion — the NEXT layer's dense xl/xr matmuls and that block's
chunked AllGather, so dense+collective stream during the edge phase
instead of serializing at the layer boundary (xr, xl_stage, xl_full are
double-buffered to make that legal).  Layer 2's completions run the
output heads instead.
"""

import sys

sys.path.insert(0, "/opt/trn_rl_repo")

import numpy as np

import concourse.bass as bass
import concourse.bacc as bacc
import concourse.tile as tile
import concourse.mybir as mybir
from concourse import bass_utils

# problem constants (hardcoded per spec)
N = 10000
IN_DIM = 512
H, C = 4, 64
HC = 256
HID = 64
NCORES = 8
NPC = N // NCORES          # 1250 nodes per core
NBLK = 10                  # 128-dst blocks per core (1280 padded)
NPAD = NBLK * 128
CH = 8                     # edge tiles per gather chunk (8*128 = 1024 idxs)
EPS = 1e-5

F32 = mybir.dt.float32
BF16 = mybir.dt.bfloat16
FP8 = mybir.dt.float8e4
NPBF = mybir.dt.np(BF16)
NPF8 = mybir.dt.np(FP8)

_cache = {}


# ---------------------------------------------------------------- host prep

def _prep_edges(edge_index):
    """Shard edges by dst range, sort by dst, tile into 128-edge tiles
    grouped by 128-dst blocks, pad so every core has identical tile
    structure.  Returns (meta, per_core)."""
    src = np.asarray(edge_index[0], dtype=np.int64)
    dst = np.asarray(edge_index[1], dtype=np.int64)

    per_core_edges = []
    counts = np.zeros((NCORES, NBLK), dtype=np.int64)
    for k in range(NCORES):
        sel = (dst // NPC) == k
        s = src[sel]
        d = dst[sel] - k * NPC
        order = np.argsort(d, kind="stable")
        s, d = s[order], d[order]
        per_core_edges.append((s, d))
        bid = d // 128
        for b in range(NBLK):
            counts[k, b] = int((bid == b).sum())

    T_b = [int(np.ceil(counts[:, b].max() / 128)) for b in range(NBLK)]
    tot = sum(T_b)
    T_b[-1] += (-tot) % CH
    T = sum(T_b)
    base_b = np.cumsum([0] + T_b[:-1])

    per_core = []
    for k in range(NCORES):
        s, d = per_core_edges[k]
        # remap src to the block-major xl_full layout (AllGather outputs must
        # be contiguous): row = blk(g)*NCORES*128 + core(g)*128 + pos(g)
        loc = s % NPC
        s = (loc // 128) * (NCORES * 128) + (s // NPC) * 128 + (loc % 128)
        src_ids = np.zeros(T * 128, dtype=np.int16)
        S = np.zeros((128, T, 128), dtype=NPF8)
        ST = np.zeros((128, T, 128), dtype=NPF8)
        bid = d // 128
        for b in range(NBLK):
            eb = np.nonzero(bid == b)[0]
            sb, db = s[eb], d[eb]
            off = base_b[b] * 128
            j = np.arange(len(eb))
            pos = off + j
            src_ids[pos] = sb.astype(np.int16)
            S[pos % 128, pos // 128, db - 128 * b] = 1.0
            ST[db - 128 * b, pos // 128, pos % 128] = 1.0
        per_core.append({"src_ids": src_ids, "S": S, "ST": ST})

    meta = (tuple(T_b),)
    return meta, per_core


def _wrap_idx(ids):
    """[T*128] -> [128, T*8] int16 in dma_gather layout (idx e at
    partition e%16, col e//16; replicated down the 128 partitions)."""
    arr = ids.reshape(-1, 16).T.astype(np.int16)   # [16, T*8]
    return np.ascontiguousarray(np.tile(arr, (8, 1)))


def _rep(vec):
    return np.ascontiguousarray(np.tile(np.asarray(vec, np.float32).reshape(1, -1), (128, 1)))


# ---------------------------------------------------------------- device kernel

def _build(T_b):
    T = sum(T_b)
    NCHUNK = T // CH
    base_b = np.cumsum([0] + list(T_b[:-1]))
    # per-tile metadata
    blk_of = np.zeros(T, dtype=np.int64)
    first_of = np.zeros(T, dtype=bool)
    last_of = np.zeros(T, dtype=bool)
    for b in range(NBLK):
        blk_of[base_b[b]:base_b[b] + T_b[b]] = b
        first_of[base_b[b]] = True
        last_of[base_b[b] + T_b[b] - 1] = True

    nc = bacc.Bacc("TRN2", target_bir_lowering=False, debug=False,
                   enable_asserts=True, num_devices=NCORES)

    # ---- I/O
    xT_in = nc.dram_tensor("xT_in", [128, 4, NPAD], BF16, kind="ExternalInput")
    w_in = {}
    for l, K in ((0, IN_DIM), (1, HC), (2, HC)):
        w_in[f"Wl{l}"] = nc.dram_tensor(f"Wl{l}_in", [K, HC], BF16, kind="ExternalInput")
        w_in[f"Wr{l}"] = nc.dram_tensor(f"Wr{l}_in", [K, HC], BF16, kind="ExternalInput")
    p0W_in = nc.dram_tensor("p0W_in", [IN_DIM, HC], BF16, kind="ExternalInput")
    outW_in = nc.dram_tensor("outW_in", [HC, HID], BF16, kind="ExternalInput")
    visW_in = nc.dram_tensor("visW_in", [HID, HID], BF16, kind="ExternalInput")
    txtW_in = nc.dram_tensor("txtW_in", [HID, HID], BF16, kind="ExternalInput")

    attb_in = nc.dram_tensor("attb_in", [128, 3, HC], BF16, kind="ExternalInput")
    blr_in = nc.dram_tensor("blr_in", [128, 3, HC], F32, kind="ExternalInput")
    brr_in = nc.dram_tensor("brr_in", [128, 3, HC], F32, kind="ExternalInput")
    cbr_in = nc.dram_tensor("cbr_in", [128, 3, HC], F32, kind="ExternalInput")
    gr_in = nc.dram_tensor("gr_in", [128, 3, HC], F32, kind="ExternalInput")
    br2_in = nc.dram_tensor("br2_in", [128, 3, HC], F32, kind="ExternalInput")
    p0br_in = nc.dram_tensor("p0br_in", [128, HC], F32, kind="ExternalInput")
    outbr_in = nc.dram_tensor("outbr_in", [128, HID], F32, kind="ExternalInput")
    visbr_in = nc.dram_tensor("visbr_in", [128, HID], F32, kind="ExternalInput")
    txtbr_in = nc.dram_tensor("txtbr_in", [128, HID], F32, kind="ExternalInput")
    identf_in = nc.dram_tensor("identf_in", [128, 128], F32, kind="ExternalInput")
    identb_in = nc.dram_tensor("identb_in", [128, 128], BF16, kind="ExternalInput")

    isrc_in = nc.dram_tensor("isrc_in", [128, T * 8], mybir.dt.int16, kind="ExternalInput")
    S_in = nc.dram_tensor("S_in", [128, T, 128], FP8, kind="ExternalInput")
    ST_in = nc.dram_tensor("ST_in", [128, T, 128], FP8, kind="ExternalInput")

    o_out = nc.dram_tensor("o_out", [NPC, HID], F32, kind="ExternalOutput")
    v_out = nc.dram_tensor("v_out", [NPC, HID], F32, kind="ExternalOutput")
    t_out = nc.dram_tensor("t_out", [NPC, HID], F32, kind="ExternalOutput")

    # internal DRAM (double-buffered per layer parity)
    xl_stage = [nc.dram_tensor(f"xl_stage{p}", [NPAD, HC], BF16, kind="Internal")
                for p in range(2)]
    xl_full = [nc.dram_tensor(f"xl_full{p}", [NCORES * NPAD, HC], BF16,
                              kind="Internal", addr_space="Shared")
               for p in range(2)]

    with tile.TileContext(nc) as tc:
        with tc.tile_pool(name="const", bufs=1) as cpool, \
             tc.tile_pool(name="state", bufs=1) as spool, \
             tc.tile_pool(name="work", bufs=2) as wpool, \
             tc.tile_pool(name="chunk", bufs=4) as chpool, \
             tc.tile_pool(name="dpsum", bufs=2, space="PSUM") as dpsum, \
             tc.tile_pool(name="apsum", bufs=2, space="PSUM") as apsum, \
             tc.tile_pool(name="ypsum", bufs=2, space="PSUM") as ypsum, \
             tc.tile_pool(name="tpsum", bufs=1, space="PSUM") as tpsum:

            # ---- load constants
            def ld(pool, dram_ap, shape, dtype):
                t = pool.tile(shape, dtype, tag=dram_ap.tensor.name)
                nc.sync.dma_start(t[:], dram_ap)
                return t

            w_sb = {}
            for l, K in ((0, IN_DIM), (1, HC), (2, HC)):
                KT = K // 128
                for side in ("Wl", "Wr"):
                    ap = w_in[f"{side}{l}"].ap().rearrange("(kc p) f -> p kc f", p=128)
                    w_sb[f"{side}{l}"] = ld(cpool, ap, [128, KT, HC], BF16)
            p0W_sb = ld(cpool, p0W_in.ap().rearrange("(kc p) f -> p kc f", p=128), [128, 4, HC], BF16)
            outW_sb = ld(cpool, outW_in.ap().rearrange("(kc p) f -> p kc f", p=128), [128, 2, HID], BF16)
            visW_sb = ld(cpool, visW_in.ap(), [64, HID], BF16)
            txtW_sb = ld(cpool, txtW_in.ap(), [64, HID], BF16)
            attb_sb = ld(cpool, attb_in.ap(), [128, 3, HC], BF16)
            blr_sb = ld(cpool, blr_in.ap(), [128, 3, HC], F32)
            brr_sb = ld(cpool, brr_in.ap(), [128, 3, HC], F32)
            cbr_sb = ld(cpool, cbr_in.ap(), [128, 3, HC], F32)
            gr_sb = ld(cpool, gr_in.ap(), [128, 3, HC], F32)
            br2_sb = ld(cpool, br2_in.ap(), [128, 3, HC], F32)
            p0br_sb = ld(cpool, p0br_in.ap(), [128, HC], F32)
            outbr_sb = ld(cpool, outbr_in.ap(), [128, HID], F32)
            visbr_sb = ld(cpool, visbr_in.ap(), [128, HID], F32)
            txtbr_sb = ld(cpool, txtbr_in.ap(), [128, HID], F32)
            identf_sb = ld(cpool, identf_in.ap(), [128, 128], F32)
            identb_sb = ld(cpool, identb_in.ap(), [128, 128], BF16)
            isrc_sb = ld(cpool, isrc_in.ap(), [128, T * 8], mybir.dt.int16)
            xT_sb = ld(spool, xT_in.ap(), [128, 4, NPAD], BF16)

            hTa = spool.tile([128, 2, NPAD], BF16, tag="hTa")
            hTb = spool.tile([128, 2, NPAD], BF16, tag="hTb")
            resA = spool.tile([128, NBLK, HC], F32, tag="resA")
            resB = spool.tile([128, NBLK, HC], F32, tag="resB")
            xr_bufs = [spool.tile([128, NBLK, HC], BF16, tag=f"xr{p}",
                                  name=f"xr{p}")
                       for p in range(2)]

            def dense_block(l, rt, lhs_of, res_dst):
                """Dense xl/xr (+res) for layer l, node block rt, plus the
                chunked AllGather of that xl block.  Parity: layer l uses
                stage/full/xr index l%2."""
                p = l % 2
                K = IN_DIM if l == 0 else HC
                KT = K // 128
                nsl = slice(rt * 128, (rt + 1) * 128)
                psl = dpsum.tile([128, HC], F32, tag="dps")
                for kc in range(KT):
                    nc.tensor.matmul(psl[:], lhs_of(kc, nsl), w_sb[f"Wl{l}"][:, kc, :],
                                     start=(kc == 0), stop=(kc == KT - 1))
                xl_t = wpool.tile([128, HC], BF16, tag="xlev")
                nc.vector.tensor_add(xl_t[:], psl[:], blr_sb[:, l, :])
                nc.sync.dma_start(xl_stage[p].ap()[nsl, :], xl_t[:])
                nc.gpsimd.collective_compute(
                    "AllGather", mybir.AluOpType.bypass,
                    replica_groups=[list(range(NCORES))],
                    ins=[xl_stage[p].ap()[nsl, :]],
                    outs=[xl_full[p].ap()[rt * NCORES * 128:(rt + 1) * NCORES * 128, :]],
                )
                psr = dpsum.tile([128, HC], F32, tag="dps")
                for kc in range(KT):
                    nc.tensor.matmul(psr[:], lhs_of(kc, nsl), w_sb[f"Wr{l}"][:, kc, :],
                                     start=(kc == 0), stop=(kc == KT - 1))
                nc.vector.tensor_add(xr_bufs[p][:, rt, :], psr[:], brr_sb[:, l, :])
                if l == 0:
                    psp = dpsum.tile([128, HC], F32, tag="dps")
                    for kc in range(4):
                        nc.tensor.matmul(psp[:], xT_sb[:, kc, nsl], p0W_sb[:, kc, :],
                                         start=(kc == 0), stop=(kc == 3))
                    nc.vector.tensor_add(res_dst[:, rt, :], psp[:], p0br_sb[:])

            hT_cur, res_cur, h_next = None, resA, resB

            # ---- layer-0 dense prologue
            for rt in range(NBLK):
                dense_block(0, rt, lambda kc, nsl: xT_sb[:, kc, nsl], res_cur)

            for l in range(3):
                p = l % 2
                xr_sb = xr_bufs[p]
                hT_next = hTa if hT_cur is not hTa else hTb

                # ---- edge phase
                agg_ps = {}
                for ch in range(NCHUNK):
                    csl = slice(ch * CH * 8, (ch + 1) * CH * 8)
                    g_ch = chpool.tile([128, CH, HC], BF16, tag="gch")
                    nc.gpsimd.dma_gather(g_ch[:], xl_full[p].ap(), isrc_sb[:, csl],
                                         CH * 128, CH * 128, HC)
                    s_ch = chpool.tile([128, CH, 128], FP8, tag="sch")
                    nc.sync.dma_start(s_ch[:], S_in.ap()[:, ch * CH:(ch + 1) * CH, :])
                    st_ch = chpool.tile([128, CH, 128], FP8, tag="stch")
                    nc.sync.dma_start(st_ch[:], ST_in.ap()[:, ch * CH:(ch + 1) * CH, :])

                    # y = xr[dst] (one-hot expand) + xl[src] (identity acc), in PSUM
                    e_ch = chpool.tile([128, CH, HC], BF16, tag="ech")
                    for quarter in range(4):
                        y_ps = ypsum.tile([128, 2, HC], F32, tag="yps")
                        # start=True clears has_written for the WHOLE bank, so
                        # each slice's group must finish before the next starts
                        for j in range(2):
                            t = quarter * 2 + j
                            b = int(blk_of[ch * CH + t])
                            nc.tensor.matmul(y_ps[:, j, :], st_ch[:, t, :],
                                             xr_sb[:, b, :], start=True, stop=False)
                            nc.tensor.matmul(y_ps[:, j, :], identb_sb[:],
                                             g_ch[:, t, :], start=False, stop=True)
                        nc.scalar.activation(
                            e_ch[:, quarter * 2:(quarter + 1) * 2, :], y_ps[:],
                            mybir.ActivationFunctionType.Prelu, alpha=0.2)

                    nc.vector.tensor_tensor(
                        e_ch[:], e_ch[:],
                        attb_sb[:, l:l + 1, :].broadcast_to([128, CH, HC]),
                        mybir.AluOpType.mult)
                    # alpha via log-tree of 2x-mode TT adds (plain reduce runs at 1x)
                    al_ch = chpool.tile([128, CH * H], F32, tag="alch")
                    p3 = e_ch[:].rearrange("p t (h c) -> p (t h) c", h=H)
                    q1_ch = chpool.tile([128, CH * H, C // 2], BF16, tag="q1ch")
                    nc.vector.tensor_add(q1_ch[:], p3[:, :, 0:32], p3[:, :, 32:64])
                    q2_ch = chpool.tile([128, CH * H, C // 4], BF16, tag="q2ch")
                    nc.vector.tensor_add(q2_ch[:], q1_ch[:, :, 0:16], q1_ch[:, :, 16:32])
                    q3_ch = chpool.tile([128, CH * H, C // 8], BF16, tag="q3ch")
                    nc.vector.tensor_add(q3_ch[:], q2_ch[:, :, 0:8], q2_ch[:, :, 8:16])
                    nc.vector.tensor_reduce(
                        al_ch[:], q3_ch[:],
                        axis=mybir.AxisListType.X, op=mybir.AluOpType.add)

                    # exp broadcast-expanded on ACT, then 2x in-place multiply by G
                    va_ch = chpool.tile([128, CH, HC + H], BF16, tag="vach")
                    nc.scalar.activation(
                        va_ch[:, :, 0:HC].rearrange("p t (h c) -> p t h c", h=H),
                        al_ch[:].rearrange("p (t h o) -> p t h o", h=H, o=1)
                            .broadcast_to([128, CH, H, C]),
                        mybir.ActivationFunctionType.Exp)
                    nc.scalar.activation(
                        va_ch[:, :, HC:HC + H],
                        al_ch[:].rearrange("p (t h) -> p t h", h=H),
                        mybir.ActivationFunctionType.Exp)
                    nc.vector.tensor_tensor(
                        va_ch[:, :, 0:HC], g_ch[:], va_ch[:, :, 0:HC],
                        mybir.AluOpType.mult)

                    for t in range(CH):
                        gt = ch * CH + t
                        b = int(blk_of[gt])
                        if first_of[gt]:
                            agg_ps[b] = apsum.tile([128, HC + H], F32, tag="aggps",
                                                   name=f"aggps_l{l}_b{b}")
                        nc.tensor.matmul(agg_ps[b][:], s_ch[:, t, :], va_ch[:, t, :],
                                         start=bool(first_of[gt]), stop=bool(last_of[gt]))

                        if not last_of[gt]:
                            continue
                        ps = agg_ps.pop(b)
                        # softmax normalize + conv bias + residual
                        rec_t = wpool.tile([128, H], F32, tag="rec")
                        nc.vector.reciprocal(rec_t[:], ps[:, HC:HC + H])
                        h3_t = wpool.tile([128, HC], F32, tag="h3")
                        nc.vector.tensor_tensor(
                            h3_t[:].rearrange("p (h c) -> p h c", h=H),
                            ps[:, 0:HC].rearrange("p (h c) -> p h c", h=H),
                            rec_t[:].rearrange("p (h c) -> p h c", c=1)
                                .broadcast_to([128, H, C]),
                            mybir.AluOpType.mult)
                        nc.vector.tensor_add(h3_t[:], h3_t[:], cbr_sb[:, l, :])
                        nc.vector.tensor_add(h3_t[:], h3_t[:], res_cur[:, b, :])
                        # layernorm
                        mu_t = wpool.tile([128, 1], F32, tag="mu")
                        nc.vector.tensor_reduce(mu_t[:], h3_t[:],
                                                axis=mybir.AxisListType.X,
                                                op=mybir.AluOpType.add)
                        nmu_t = wpool.tile([128, 1], F32, tag="nmu")
                        nc.vector.tensor_scalar_mul(nmu_t[:], mu_t[:], -1.0 / HC)
                        xc_t = wpool.tile([128, HC], F32, tag="xc")
                        nc.scalar.activation(xc_t[:], h3_t[:],
                                             mybir.ActivationFunctionType.Identity,
                                             bias=nmu_t[:])
                        sq_t = wpool.tile([128, HC], F32, tag="sq")
                        var_t = wpool.tile([128, 1], F32, tag="var")
                        nc.scalar.activation(sq_t[:], xc_t[:],
                                             mybir.ActivationFunctionType.Square,
                                             accum_out=var_t[:])
                        ve_t = wpool.tile([128, 1], F32, tag="ve")
                        nc.vector.tensor_scalar(ve_t[:], var_t[:], 1.0 / HC, EPS,
                                                mybir.AluOpType.mult,
                                                mybir.AluOpType.add)
                        # rsqrt via magic-constant + 2 Newton iterations (DVE only;
                        # avoids ACT table switches for Ln/Sqrt)
                        vi_t = wpool.tile([128, 1], mybir.dt.int32, tag="vi")
                        nc.vector.tensor_scalar(
                            vi_t[:], ve_t[:].bitcast(mybir.dt.int32), 1, None,
                            mybir.AluOpType.logical_shift_right)
                        nc.vector.tensor_scalar(vi_t[:], vi_t[:], -1, 0x5f3759df,
                                                mybir.AluOpType.mult,
                                                mybir.AluOpType.add)
                        rstd_t = wpool.tile([128, 1], F32, tag="rstd")
                        y_t = vi_t[:].bitcast(F32)
                        t1_t = wpool.tile([128, 1], F32, tag="t1")
                        for _it in range(2):
                            nc.vector.tensor_mul(t1_t[:], y_t, y_t)
                            nc.vector.tensor_mul(t1_t[:], t1_t[:], ve_t[:])
                            nc.vector.tensor_scalar(t1_t[:], t1_t[:], -0.5, 1.5,
                                                    mybir.AluOpType.mult,
                                                    mybir.AluOpType.add)
                            if _it == 0:
                                nc.vector.tensor_mul(vi_t[:].bitcast(F32), y_t, t1_t[:])
                            else:
                                nc.vector.tensor_mul(rstd_t[:], y_t, t1_t[:])
                        xn_t = wpool.tile([128, HC], F32, tag="xn")
                        nc.scalar.activation(xn_t[:], xc_t[:],
                                             mybir.ActivationFunctionType.Identity,
                                             scale=rstd_t[:])
                        nc.vector.tensor_tensor(xn_t[:], xn_t[:], gr_sb[:, l, :],
                                                mybir.AluOpType.mult)
                        nc.vector.tensor_add(xn_t[:], xn_t[:], br2_sb[:, l, :])
                        # elu
                        mn_t = wpool.tile([128, HC], F32, tag="mn")
                        nc.vector.tensor_scalar_min(mn_t[:], xn_t[:], 0.0)
                        ex_t = wpool.tile([128, HC], F32, tag="ex")
                        nc.scalar.activation(ex_t[:], mn_t[:],
                                             mybir.ActivationFunctionType.Exp)
                        rm_t = wpool.tile([128, HC], F32, tag="rm")
                        nc.vector.tensor_scalar(rm_t[:], xn_t[:], 0.0, -1.0,
                                                mybir.AluOpType.max,
                                                mybir.AluOpType.add)
                        nc.vector.tensor_add(h_next[:, b, :], ex_t[:], rm_t[:])
                        # transpose into next layer's hT (bf16 cast on ACT copy)
                        for cc in range(2):
                            tp_ps = tpsum.tile([128, 128], F32, tag="tp")
                            nc.tensor.transpose(
                                tp_ps[:], h_next[:, b, cc * 128:(cc + 1) * 128],
                                identf_sb[:])
                            nc.scalar.activation(
                                hT_next[:, cc, b * 128:(b + 1) * 128], tp_ps[:],
                                mybir.ActivationFunctionType.Copy)

                        if l < 2:
                            # next layer's dense for this block + chunked AG,
                            # streamed during this layer's edge phase
                            dense_block(l + 1, b,
                                        lambda kc, nsl, _h=hT_next: _h[:, kc, nsl],
                                        None)
                        else:
                            # ---- final heads per block
                            rows = NPC - b * 128 if b == NBLK - 1 else 128
                            nsl2 = slice(b * 128, (b + 1) * 128)
                            pso = dpsum.tile([128, HID], F32, tag="dps")
                            for kc in range(2):
                                nc.tensor.matmul(pso[:], hT_next[:, kc, nsl2],
                                                 outW_sb[:, kc, :],
                                                 start=(kc == 0), stop=(kc == 1))
                            o_t = wpool.tile([128, HID], F32, tag="oev")
                            nc.vector.tensor_add(o_t[:], pso[:], outbr_sb[:])
                            nc.sync.dma_start(
                                o_out.ap()[b * 128: b * 128 + rows, :], o_t[0:rows, :])

                            tp2_ps = tpsum.tile([128, 128], F32, tag="tp")
                            nc.tensor.transpose(tp2_ps[0:64, :], o_t[:], identf_sb[:])
                            oT_t = wpool.tile([64, 128], BF16, tag="oT")
                            nc.scalar.activation(oT_t[:], tp2_ps[0:64, :],
                                                 mybir.ActivationFunctionType.Copy)

                            for W_sb, b_sb, dst_dram in ((visW_sb, visbr_sb, v_out),
                                                         (txtW_sb, txtbr_sb, t_out)):
                                psv = dpsum.tile([128, HID], F32, tag="dps")
                                nc.tensor.matmul(psv[:], oT_t[:], W_sb[:],
                                                 start=True, stop=True)
                                vb_t = wpool.tile([128, HID], F32, tag="vbev")
                                nc.vector.tensor_add(vb_t[:], psv[:], b_sb[:])
                                vr_t = wpool.tile([128, HID], F32, tag="vrev")
                                nc.scalar.activation(vr_t[:], vb_t[:],
                                                     mybir.ActivationFunctionType.Relu)
                                nc.sync.dma_start(
                                    dst_dram.ap()[b * 128: b * 128 + rows, :],
                                    vr_t[0:rows, :])

                hT_cur = hT_next
                res_cur, h_next = h_next, res_cur

    nc.compile()
    return nc


# ---------------------------------------------------------------- entry point

def prepare(**inputs):
    x = np.asarray(inputs["x"], np.float32)
    edge_index = np.asarray(inputs["edge_index"])

    meta, per_core = _prep_edges(edge_index)
    (T_b,) = meta
    T = sum(T_b)

    key = meta
    if key not in _cache:
        _cache[key] = _build(T_b)
    nc = _cache[key]

    # shared (replicated) inputs
    shared = {}
    for l in range(3):
        shared[f"Wl{l}_in"] = np.asarray(inputs[f"Wl{l}"], np.float32).astype(NPBF)
        shared[f"Wr{l}_in"] = np.asarray(inputs[f"Wr{l}"], np.float32).astype(NPBF)
    shared["p0W_in"] = np.asarray(inputs["proj0_W"], np.float32).astype(NPBF)
    shared["outW_in"] = np.asarray(inputs["out_W"], np.float32).astype(NPBF)
    shared["visW_in"] = np.asarray(inputs["vis_W"], np.float32).astype(NPBF)
    shared["txtW_in"] = np.asarray(inputs["txt_W"], np.float32).astype(NPBF)
    shared["attb_in"] = np.stack(
        [_rep(np.asarray(inputs[f"att{l}"], np.float32).reshape(-1)) for l in range(3)],
        axis=1).astype(NPBF)
    shared["blr_in"] = np.stack([_rep(inputs[f"bl{l}"]) for l in range(3)], axis=1)
    shared["brr_in"] = np.stack([_rep(inputs[f"br{l}"]) for l in range(3)], axis=1)
    shared["cbr_in"] = np.stack([_rep(inputs[f"cb{l}"]) for l in range(3)], axis=1)
    ln_g = np.asarray(inputs["ln_g"], np.float32)
    ln_b = np.asarray(inputs["ln_b"], np.float32)
    shared["gr_in"] = np.stack([_rep(ln_g[l]) for l in range(3)], axis=1)
    shared["br2_in"] = np.stack([_rep(ln_b[l]) for l in range(3)], axis=1)
    shared["p0br_in"] = _rep(inputs["proj0_b"])
    shared["outbr_in"] = _rep(inputs["out_b"])
    shared["visbr_in"] = _rep(inputs["vis_b"])
    shared["txtbr_in"] = _rep(inputs["txt_b"])
    shared["identf_in"] = np.eye(128, dtype=np.float32)
    shared["identb_in"] = np.eye(128, dtype=np.float32).astype(NPBF)

    in_maps = []
    for k in range(NCORES):
        xk = x[k * NPC:(k + 1) * NPC]
        xp = np.zeros((NPAD, IN_DIM), np.float32)
        xp[:NPC] = xk
        xT = np.ascontiguousarray(
            xp.T.reshape(4, 128, NPAD).transpose(1, 0, 2)).astype(NPBF)
        m = dict(shared)
        m["xT_in"] = xT
        m["isrc_in"] = _wrap_idx(per_core[k]["src_ids"])
        m["S_in"] = per_core[k]["S"]
        m["ST_in"] = per_core[k]["ST"]
        in_maps.append(m)
    return nc, in_maps


def kernel(**inputs):
    nc, in_maps = prepare(**inputs)
    import os as _os
    _trace = bool(int(_os.environ.get("BASS_KERNEL_TRACE", "0")))
    res = bass_utils.run_bass_kernel_spmd(nc, in_maps, core_ids=list(range(NCORES)),
                                          trace=_trace)

    out = np.concatenate([res.results[k]["o_out"] for k in range(NCORES)], axis=0)
    vis = np.concatenate([res.results[k]["v_out"] for k in range(NCORES)], axis=0)
    txt = np.concatenate([res.results[k]["t_out"] for k in range(NCORES)], axis=0)
    kernel.last_exec_time_ns = res.exec_time_ns
    return out, vis, txt
